# revision 60
# baseline (speedup 1.0000x reference)
"""Trainium2 Bass kernel for nn_DecoderLayer_19851338842283.

8 cores: data-parallel over batch (4) x tensor-parallel (2) over heads/mlp_dim.
fp8(e4m3) DoubleRow matmuls for projections / MLP (with host-side residual
weight passes for accuracy), fp8-DR d-split scores, bf16 exp/ctx/out-proj.
Host sums the two tensor-parallel partials and adds the residual.

Scheduling/structure (402.7us -> 307.4us on the TimelineSim cost model):
- consolidated DMAs (few big transfers, weights interleaved with x quarters
  so the PE starts ~5us in instead of ~45us)
- q/k projections at 2 fp8 passes (s1@x8 + s3@xr)
- rel-pos band applied as exp(s)*exp(b) on the Pool engine (off PE/DVE)
- two k-tiles of scores share a 2-bank psum tile so one exp instruction
  covers both (halves the Act per-instruction overhead)
- ctx computed in [q, d+1] layout (65-col moving operand: 2.3x less PE time
  than the [d, q] orientation), denominator via the ones column; per-head
  qc-major drains keep psum accumulation groups contiguous per bank (HW
  accumulation state is bank-level - interleaved groups corrupt)
- normalized ctx transposed back to [d, q] with PE transposes (2 heads per
  transpose), deferred a few units to stay off the DVE critical path
- producer (scores/exp/band) runs a full head ahead of the ctx drain;
  MLP-in units interleave into attention at a fixed cadence
- bf16 outputs, halved output DMA traffic
"""

import ml_dtypes
import numpy as np

import concourse.bacc as bacc
import concourse.mybir as mybir
import concourse.tile as tile
from concourse.bass_utils import run_bass_kernel_spmd

F32 = mybir.dt.float32
BF16 = mybir.dt.bfloat16
FP8 = mybir.dt.float8e4
Act = mybir.ActivationFunctionType
Alu = mybir.AluOpType
DR = mybir.MatmulPerfMode.DoubleRow
E4 = ml_dtypes.float8_e4m3
BF = ml_dtypes.bfloat16

B, L, E, H, D, F = 4, 2048, 1024, 16, 64, 4096
HC = H // 2          # heads per core = 8
FC = F // 2          # mlp dim per core = 2048
NCORES = 8
ET = E // 128        # 8
LT = L // 128        # 16
FT = FC // 128       # 16
NSUP = L // 512      # 4
TB = E // 256        # 4 DR pair-blocks over E
PB = FC // 256       # 8 DR pair-blocks over FC
BAND_OFF = 128
BAND_W = 384
NUM_BUCKETS = 32
QSC = np.float32(0.125 ** 0.5 / 16.0)   # psum->q8/k8 copy scale


def _build(causal: bool):
    nc = bacc.Bacc("TRN2", target_bir_lowering=False, debug=False,
                   num_devices=NCORES)
    x8_d = nc.dram_tensor("x8", [128, ET, L], FP8, kind="ExternalInput").ap()
    xr_d = nc.dram_tensor("xr", [128, ET, L], FP8, kind="ExternalInput").ap()
    # q/k weights: [tl, part, ps(2: s1|s3), tb, 2, 128]
    wqs_d = nc.dram_tensor("wqs", [4, 128, 2, TB, 2, 128], FP8,
                           kind="ExternalInput").ap()
    wks_d = nc.dram_tensor("wks", [4, 128, 2, TB, 2, 128], FP8,
                           kind="ExternalInput").ap()
    # v weights: [vh, part, ps(3), tb, 2, 256]
    wvm_d = nc.dram_tensor("wvm", [2, 128, 3, TB, 2, 256], FP8,
                           kind="ExternalInput").ap()
    # wi weights: [ftpair, part, j(2), ps(3), tb, 2, 128]
    wis_d = nc.dram_tensor("wis", [FT // 2, 128, 2, 3, TB, 2, 128], FP8,
                           kind="ExternalInput").ap()
    wmm_d = nc.dram_tensor("wmm", [2, 128, FT, E], FP8,
                           kind="ExternalInput").ap()
    wos_d = nc.dram_tensor("wos", [128, 4, E], BF16, kind="ExternalInput").ap()
    band_d = nc.dram_tensor("band", [128, HC, BAND_W], BF16,
                            kind="ExternalInput").ap()
    ident_d = nc.dram_tensor("ident", [128, 128], BF16,
                             kind="ExternalInput").ap()
    bfut_d = nc.dram_tensor("bfut", [128, HC], F32, kind="ExternalInput").ap()
    attn_d = nc.dram_tensor("attn_out", [L, E], BF16,
                            kind="ExternalOutput").ap()
    mlp_d = nc.dram_tensor("mlp_out", [L, E], BF16,
                           kind="ExternalOutput").ap()

    with tile.TileContext(nc) as tc:
        with (
            tc.tile_pool(name="pbig", bufs=1) as pbig,
            tc.tile_pool(name="pqk", bufs=4) as pqk,
            tc.tile_pool(name="pva", bufs=1) as pva,
        ):
            x8 = pbig.tile([128, ET, L], FP8, tag="x8", name="x8")
            wms = [pbig.tile([128, FT, E], FP8, tag=f"wm{ps}", name=f"wm{ps}")
                   for ps in range(2)]
            xr = pbig.tile([128, ET, L], FP8, tag="xr", name="xr")
            h8 = pbig.tile([128, FT, L], FP8, tag="h8", name="h8")
            q8s = [pqk.tile([128, 2, L], FP8, tag="qk", name=f"q8_{g}")
                   for g in range(2)]
            k8s = [pqk.tile([128, 2, L], FP8, tag="qk", name=f"k8_{g}")
                   for g in range(2)]
            va_all = pva.tile([128, LT * HC * 65], BF16, tag="va", name="va")

            # ---------------- q/k/v projections -----------------------------
            with (
                tc.tile_pool(name="pw", bufs=10) as pw,
                tc.tile_pool(name="pps1", bufs=3, space="PSUM") as pps1,
            ):
                # Allocate all stationary-weight tiles, then issue DMAs in an
                # order that lets the PE start as early as possible: first
                # q-weight tile, then x quarters interleaved with the
                # remaining weight tiles.
                sts_q = [pw.tile([128, 2, TB, 2, 128], FP8, tag="w",
                                 name=f"stq{tl}") for tl in range(4)]
                sts_k = [pw.tile([128, 2, TB, 2, 128], FP8, tag="w",
                                 name=f"stk{tl}") for tl in range(4)]
                wvs = [pw.tile([128, 3, TB, 2, 256], FP8, tag="w",
                               name=f"wv{vh}") for vh in range(2)]

                def xq(c):
                    nc.sync.dma_start(x8[:, :, 512 * c:512 * c + 512],
                                      x8_d[:, :, 512 * c:512 * c + 512])
                    nc.sync.dma_start(xr[:, :, 512 * c:512 * c + 512],
                                      xr_d[:, :, 512 * c:512 * c + 512])

                nc.sync.dma_start(sts_q[0][:], wqs_d[0])
                xq(0)
                nc.sync.dma_start(sts_k[0][:], wks_d[0])
                for tl in range(1, 4):
                    nc.sync.dma_start(sts_q[tl][:], wqs_d[tl])
                    nc.sync.dma_start(sts_k[tl][:], wks_d[tl])
                for c in range(1, NSUP):
                    xq(c)
                for vh in range(2):
                    nc.sync.dma_start(wvs[vh][:], wvm_d[vh])
                for ps in range(2):
                    nc.sync.dma_start(wms[ps][:], wmm_d[ps])

                # q/k: 2 passes (s1 @ x8 + s3 @ xr)
                def xmov2(ps, tb, c0, cw):
                    src = x8 if ps == 0 else xr
                    return src[:, 2 * tb:2 * tb + 2, c0:c0 + cw]

                # v / wi: 3 passes (s1 @ x8 + s2 @ x8 + s3 @ xr)
                def xmov3(ps, tb, c0, cw):
                    src = x8 if ps < 2 else xr
                    return src[:, 2 * tb:2 * tb + 2, c0:c0 + cw]

                # c-major so the PE consumes x quarters in DMA-arrival order;
                # q/k alternated to match the weight-DMA issue order
                for c in range(NSUP):
                    for tl in range(4):          # (g, dpair)
                        for sts, dsts in ((sts_q, q8s), (sts_k, k8s)):
                            g, dp = divmod(tl, 2)
                            st = sts[tl]
                            acc = pps1.tile([128, 512], F32, tag="ps1")
                            for ps in range(2):
                                for tb in range(TB):
                                    nc.tensor.matmul(
                                        acc[:], st[:, ps, tb],
                                        xmov2(ps, tb, 512 * c, 512),
                                        start=(ps == 0 and tb == 0),
                                        stop=(ps == 1 and tb == TB - 1),
                                        perf_mode=DR)
                            nc.scalar.mul(
                                dsts[g][:, dp, 512 * c:512 * c + 512],
                                acc[:], float(QSC))

                # ---------------- v projection (bf16 va + ones) -------------
                ones_c = nc.const_aps.tensor(1.0, [128, HC, 1], BF16)
                for vh in range(2):
                    wv = wvs[vh]
                    for lt in range(LT):
                        acc = pps1.tile([128, 256], F32, tag="ps1")
                        for ps in range(3):
                            for tb in range(TB):
                                nc.tensor.matmul(
                                    acc[:], xmov3(ps, tb, 128 * lt, 128),
                                    wv[:, ps, tb],
                                    start=(ps == 0 and tb == 0),
                                    stop=(ps == 2 and tb == TB - 1),
                                    perf_mode=DR)
                        va3 = va_all[:, 520 * lt:520 * lt + 520] \
                            .rearrange("p (h c) -> p h c", h=HC)
                        nc.vector.tensor_scalar_mul(
                            va3[:, 4 * vh:4 * vh + 4, 0:64],
                            acc[:].rearrange("p (h c) -> p h c", h=4),
                            1.0 / 16.0)
                        if vh == 0:
                            nc.vector.tensor_copy(va3[:, :, 64:65], ones_c)

            # -------- attention (+ interleaved MLP-in) ----------------------
            with (
                tc.tile_pool(name="pwi", bufs=3) as pwi,
                tc.tile_pool(name="pband", bufs=1) as pband,
                tc.tile_pool(name="pct", bufs=12) as pct,
                tc.tile_pool(name="pwo", bufs=1) as pwo,
                tc.tile_pool(name="pexp", bufs=13) as pexp,
                tc.tile_pool(name="prr", bufs=2) as prr,
                tc.tile_pool(name="pcsb", bufs=3) as pcsb,
                tc.tile_pool(name="poba", bufs=2) as poba,
                tc.tile_pool(name="pps", bufs=2, space="PSUM") as pps,
                tc.tile_pool(name="pctx", bufs=1, space="PSUM") as pctx,
                tc.tile_pool(name="pasm", bufs=1, space="PSUM") as pasm,
                tc.tile_pool(name="pout", bufs=1, space="PSUM") as pout,
                tc.tile_pool(name="pmps", bufs=1, space="PSUM") as pmps,
            ):
                band_sb = pband.tile([128, HC * BAND_W], BF16, tag="band")
                band3 = band_sb[:].rearrange("p (h w) -> p h w", h=HC)
                nc.sync.dma_start(band3, band_d[:, :, :])
                ident = pband.tile([128, 128], BF16, tag="ident")
                nc.sync.dma_start(ident[:], ident_d)
                bfut_sb = pband.tile([128, HC], F32, tag="bfut")
                if not causal:
                    nc.sync.dma_start(bfut_sb[:], bfut_d)
                wos = pwo.tile([128, 4, E], BF16, tag="wo", name="wos")
                nc.sync.dma_start(wos[:], wos_d)

                mlp_units = [(ft, c) for ft in range(FT) for c in range(NSUP)]
                mo_units = [(lt, ec) for lt in range(LT) for ec in range(2)]
                h4a = h8[:].rearrange("p (j pb) l -> p j pb l", j=2)
                wm4a = [w[:].rearrange("p (j pb) e -> p j pb e", j=2)
                        for w in wms]

                mlp_i = 0
                wi_sts = {}

                def wi_fetch(fp):
                    if fp >= FT // 2 or fp in wi_sts:
                        return
                    t = pwi.tile([128, 2, 3, TB, 2, 128], FP8, tag="wi",
                                 name=f"wist{fp}")
                    nc.sync.dma_start(t[:], wis_d[fp])
                    wi_sts[fp] = t

                wi_fetch(0)
                wi_fetch(1)
                wi_fetch(2)

                def emit_mlp_unit():
                    nonlocal mlp_i
                    if mlp_i >= len(mlp_units):
                        return
                    ft, c = mlp_units[mlp_i]
                    mlp_i += 1
                    if c == 0:
                        wi_fetch(ft // 2 + 2)
                    t = wi_sts[ft // 2]
                    acc = pmps.tile([128, 512], F32, tag="mps")
                    for ps in range(3):
                        for tb in range(TB):
                            nc.tensor.matmul(
                                acc[:], t[:, ft % 2, ps, tb],
                                xmov3(ps, tb, 512 * c, 512),
                                start=(ps == 0 and tb == 0),
                                stop=(ps == 2 and tb == TB - 1),
                                perf_mode=DR)
                    nc.vector.tensor_scalar(
                        out=h8[:, ft, 512 * c:512 * c + 512], in0=acc[:],
                        scalar1=1.0 / 16.0, scalar2=0.0,
                        op0=Alu.mult, op1=Alu.max)

                ob_tiles = {}

                def outproj(s_prev, cts_prev, chunks, last=False):
                    for qt, ec in chunks:
                        acc = pout.tile([128, 512], F32, tag="out",
                                        name="opacc")
                        for p in range(4):
                            nc.tensor.matmul(
                                acc[:],
                                cts_prev[p][:, 128 * qt:128 * qt + 128],
                                wos[:, p, 512 * ec:512 * ec + 512],
                                start=(p == 0), stop=(p == 3))
                        if ec == 0:
                            ob_tiles[qt] = poba.tile([128, 1024], BF16,
                                                     tag="ob",
                                                     name=f"ob{s_prev}_{qt}")
                        ob = ob_tiles[qt]
                        if last and ec == 1 and qt == 3:
                            nc.scalar.copy(ob[:, 512 * ec:512 * ec + 512],
                                           acc[:])
                        else:
                            nc.vector.tensor_copy(
                                ob[:, 512 * ec:512 * ec + 512], acc[:])
                        if ec == 1:
                            qs0 = 512 * s_prev
                            nc.sync.dma_start(
                                attn_d[qs0 + 128 * qt:qs0 + 128 * qt + 128,
                                       :], ob[:])

                def nkt(s):
                    return 4 * (s + 1) if causal else LT

                cts_map = {}
                csb_map = {}
                ea_map = {}
                ktc = [0]

                pending = []     # deferred transpose+copy blocks

                def normalize(s, cps, h):
                    # cps [128, 4, 65]: per-chunk scale by 1/denominator
                    p = h // 2
                    if h % 2 == 0:
                        csb_map[(s, p)] = pcsb.tile([128, 4, 2, 64], BF16,
                                                    tag="csb",
                                                    name=f"csb{s}_{p}")
                    csb = csb_map[(s, p)]
                    rr = prr.tile([128, 4, 1], F32, tag="rr", name="rr")
                    nc.vector.reciprocal(rr[:], cps[:, :, 64:65])
                    for qc in range(4):
                        nc.vector.tensor_scalar_mul(
                            csb[:, qc, h % 2, :], cps[:, qc, 0:64],
                            rr[:, qc])
                    if h % 2 == 1:
                        # both heads of p normalized: defer the PE transposes
                        # so they don't stall on the DVE normalize above
                        pending.append((s, p, csb, cts_map[s]))
                        del csb_map[(s, p)]

                def finish_pair(force=False):
                    # depth-1 deferral only: outproj chunks read all four
                    # cts[p] tiles, so the last pair's transposes must land
                    # before the next supertile's first outproj chunk
                    if not pending:
                        return
                    s, p, csb, cts = pending.pop(0)
                    asm = pasm.tile([128, 512], BF16, tag="asm",
                                    name=f"asm{s}_{p}")
                    for qc in range(4):
                        nc.tensor.transpose(
                            asm[:, 128 * qc:128 * qc + 128],
                            csb[:, qc, :, :].rearrange("p a b -> p (a b)"),
                            ident[:])
                    nc.vector.tensor_copy(cts[p][:], asm[:])

                def produce_pair(s, h, kt0):
                    # two k-tiles (kt0, kt0+1) share one 2-bank psum tile so
                    # a single exp instruction covers both (halving the Act
                    # per-instruction overhead); the diagonal pairs keep two
                    # exps to skip the masked region
                    qs = 512 * s
                    g, u = divmod(h, 4)
                    psA = pps.tile([128, 2, 512], F32, tag="ps", name="sc")
                    ea = pexp.tile([128, 2, 512], BF16, tag="exp", name="ea")
                    offs = []
                    for j in range(2):
                        kt = kt0 + j
                        k0 = 128 * kt
                        off = min(max(0, k0 - qs), 384) if causal else 0
                        offs.append(off)
                        nc.tensor.matmul(
                            psA[:, j, off:512],
                            k8s[g][32 * u:32 * u + 32, :, k0:k0 + 128],
                            q8s[g][32 * u:32 * u + 32, :, qs + off:qs + 512],
                            start=True, stop=True,
                            perf_mode=DR, tile_position=(32 * u, 0))
                        ul = (min(max(k0 - BAND_OFF - qs, 0), 512)
                              if not causal else 0)
                        if ul > 0:
                            nc.vector.tensor_scalar_add(
                                psA[:, j, 0:ul], psA[:, j, 0:ul],
                                bfut_sb[:, h:h + 1])
                    psF = psA[:].rearrange("p a b -> p (a b)")
                    eaF = ea[:].rearrange("p a b -> p (a b)")
                    if offs[1] <= 128:
                        # one exp instruction; for offs[1]==128 this also
                        # exps the stale gap [512, 640) whose output lands in
                        # ea columns no drain chunk ever reads
                        nc.scalar.activation(eaF[:, offs[0]:1024],
                                             psF[:, offs[0]:1024], Act.Exp)
                    else:
                        nc.scalar.activation(eaF[:, offs[0]:512],
                                             psF[:, offs[0]:512], Act.Exp)
                        nc.scalar.activation(eaF[:, 512 + offs[1]:1024],
                                             psF[:, 512 + offs[1]:1024],
                                             Act.Exp)
                    for j in range(2):
                        kt = kt0 + j
                        k0 = 128 * kt
                        off = offs[j]
                        o_lo = max(k0 - BAND_OFF, qs + off)
                        o_hi = min(k0 + 256, qs + 512)
                        if o_hi > o_lo:
                            # exp(s+b) == exp(s)*exp(b): banded rel-pos bias
                            # (and causal-mask zeros) as a Pool multiply; the
                            # head's last pair goes on DVE (lower latency)
                            # since the drain waits on it soonest
                            psl = slice(o_lo - qs, o_hi - qs)
                            bsl = slice(o_lo - (k0 - BAND_OFF),
                                        o_hi - (k0 - BAND_OFF))
                            eng = (nc.vector if kt0 + 2 >= nkt(s)
                                   else nc.gpsimd)
                            eng.tensor_tensor(
                                ea[:, j, psl], ea[:, j, psl],
                                band3[:, h, bsl], Alu.mult)
                        ea_map[(s, h, kt)] = (ea[:, j], off)
                        ktc[0] += 1
                        if ktc[0] % 5 == 0:
                            emit_mlp_unit()

                def consume_head(s, h):
                    # drain the whole head qc-major: each query-chunk's psum
                    # accumulation group is contiguous (HW accumulate state
                    # is per-bank, so groups must not interleave)
                    cps = pctx.tile([128, 4, 65], F32, tag="ctx",
                                    name=f"cps{s}_{h}")
                    kmax = nkt(s) - 1
                    for qc in range(4):
                        lastk = min(kmax, 4 * s + qc) if causal else kmax
                        for kt in range(lastk + 1):
                            ea, off = ea_map[(s, h, kt)]
                            nc.tensor.matmul(
                                cps[:, qc, :],
                                ea[:, 128 * qc:128 * qc + 128],
                                va_all[:, 520 * kt + 65 * h:
                                       520 * kt + 65 * h + 65],
                                start=(kt == 0), stop=(kt == lastk))
                    for kt in range(nkt(s)):
                        ea_map.pop((s, h, kt))
                    normalize(s, cps, h)
                    if s > 0:
                        outproj(s - 1, cts_map[s - 1], [divmod(h, 2)])
                        if h == HC - 1:
                            del cts_map[s - 1]

                heads = [(s, h) for s in range(NSUP) for h in range(HC)]
                for j, (s, h) in enumerate(heads):
                    if h == 0:
                        cts_map[s] = [pct.tile([128, 512], BF16, tag="ct",
                                               name=f"ct{s}_{p}")
                                      for p in range(4)]
                    for kt0 in range(0, nkt(s), 2):
                        produce_pair(s, h, kt0)
                        if kt0 == 2:
                            finish_pair()
                            if j >= 1:
                                consume_head(*heads[j - 1])
                consume_head(*heads[-1])
                while pending:
                    finish_pair(force=True)
                outproj(NSUP - 1, cts_map[NSUP - 1],
                        [(qt, ec) for qt in range(4) for ec in range(2)],
                        last=True)
                while mlp_i < len(mlp_units):
                    emit_mlp_unit()

            # ---------------- MLP down-projection ---------------------------
            with (
                tc.tile_pool(name="pobb", bufs=2) as pobb,
                tc.tile_pool(name="pps3", bufs=3, space="PSUM") as pps3,
            ):
                h4 = h8[:].rearrange("p (j pb) l -> p j pb l", j=2)
                wm4 = [w[:].rearrange("p (j pb) e -> p j pb e", j=2)
                       for w in wms]
                for lt in range(LT):
                    ob = pobb.tile([128, 1024], BF16, tag="ob")
                    for ec in range(2):
                        acc = pps3.tile([128, 512], F32, tag="ps3")
                        for ps in range(2):
                            for pb in range(PB):
                                nc.tensor.matmul(
                                    acc[:],
                                    h4[:, :, pb, 128 * lt:128 * lt + 128],
                                    wm4[ps][:, :, pb,
                                            512 * ec:512 * ec + 512],
                                    start=(ps == 0 and pb == 0),
                                    stop=(ps == 1 and pb == PB - 1),
                                    perf_mode=DR)
                        nc.scalar.mul(ob[:, 512 * ec:512 * ec + 512],
                                      acc[:], 1.0 / 32.0)
                        nc.sync.dma_start(
                            mlp_d[128 * lt:128 * lt + 128,
                                  512 * ec:512 * ec + 512],
                            ob[:, 512 * ec:512 * ec + 512])

    nc.compile()
    return nc


_NC_CACHE = {}


def _get_nc(causal: bool):
    if causal not in _NC_CACHE:
        _NC_CACHE[causal] = _build(causal)
    return _NC_CACHE[causal]


def _bucket(n):
    n = np.asarray(n)
    nf = np.maximum(n.astype(np.float32), np.float32(1.0))
    v = np.log(nf / np.float32(16.0)).astype(np.float32)
    v = (v / np.float32(np.log(8.0))) * np.float32(16.0)
    val_large = np.minimum(16 + v.astype(np.int32), NUM_BUCKETS - 1)
    return np.where(n < 16, n, val_large)


def _make_band(rel_emb, heads, causal):
    """exp() of the banded rel-pos bias (causal-masked entries -> 0)."""
    d = np.arange(-(BAND_OFF + 127), 256)
    pos = np.maximum(d, 0)
    bv = rel_emb[_bucket(pos)][:, heads] - rel_emb[NUM_BUCKETS - 1][heads]
    bv = np.where(d[:, None] >= 113, np.float32(0.0), bv)
    bv = np.exp(bv).astype(np.float32)
    if causal:
        bv = np.where(d[:, None] < 0, np.float32(0.0), bv)
    else:
        fut = np.exp(rel_emb[0][heads] - rel_emb[NUM_BUCKETS - 1][heads])
        bv = np.where(d[:, None] < 0, fut[None, :], bv)
    i = np.arange(128)[:, None]
    j = np.arange(BAND_W)[None, :]
    idx = (j - BAND_OFF - i) + (BAND_OFF + 127)
    return bv.astype(np.float32)[idx]          # [128, BAND_W, HC]


def _f8(a):
    return np.ascontiguousarray(a, dtype=np.float32).astype(E4)


def _split16(w, s):
    """-> (e4m3(s*w), e4m3(s*w - f32(e4m3(s*w))), e4m3(f32(e4m3(s*w))/s))"""
    w = np.asarray(w, np.float32)
    s1 = _f8(s * w)
    f1 = s1.astype(np.float32)
    s2 = _f8(s * w - f1)
    s3 = _f8(f1 / s)
    return s1, s2, s3


def _stat_qk(w_c):
    """w_c [E, HC, D] -> [4(tile), 128, TB, 2, 128] in f32 (pre-quant)."""
    arr = w_c.reshape(E, 2, 4, 2, 32)           # e, g, u, dp, dm
    out = np.empty((4, TB, 128, 2, 128), np.float32)
    for tl in range(4):
        g, dp = divmod(tl, 2)
        M = arr[:, g, :, dp, :].reshape(E, 128)  # m = 32u + dm
        out[tl] = M.reshape(TB, 2, 128, 128).transpose(0, 2, 1, 3)
    return out.transpose(0, 2, 1, 3, 4)          # [4, 128, TB, 2, 128]


def _prep_in_maps(inputs, wq, wk, wv, wo, wi, wmo, rel_emb, decoder_mask):
    inputs = np.asarray(inputs, dtype=np.float32)
    wq = np.asarray(wq, dtype=np.float32)
    wk = np.asarray(wk, dtype=np.float32)
    wv = np.asarray(wv, dtype=np.float32)
    wo = np.asarray(wo, dtype=np.float32)
    wi = np.asarray(wi, dtype=np.float32)
    wmo = np.asarray(wmo, dtype=np.float32)
    rel_emb = np.asarray(rel_emb, dtype=np.float32)
    mask = np.asarray(decoder_mask).reshape(L, L)

    tril = np.tril(np.ones((L, L), dtype=bool))
    if np.array_equal(mask, tril):
        causal = True
    elif mask.all():
        causal = False
    else:
        raise NotImplementedError("only causal or all-true masks supported")

    in_maps = []
    for c in range(NCORES):
        b, g = divmod(c, 2)
        heads = np.arange(HC * g, HC * (g + 1))
        band = _make_band(rel_emb, heads, causal)        # [128, W, HC]
        band = np.ascontiguousarray(band.transpose(0, 2, 1)).astype(BF)
        bfut = np.broadcast_to(
            (rel_emb[0][heads] - rel_emb[NUM_BUCKETS - 1][heads])
            .astype(np.float32), (128, HC)).copy()

        xT = inputs[b].T                                  # [E, L]
        x8 = _f8(xT)
        xr = _f8(16.0 * (xT - x8.astype(np.float32)))
        x8 = x8.reshape(ET, 128, L).transpose(1, 0, 2)    # [128, ET, L]
        xr = xr.reshape(ET, 128, L).transpose(1, 0, 2)

        wq_c = wq[:, heads, :]
        wk_c = wk[:, heads, :]
        # 2-pass q/k: keep only (s1, s3) -> [4, 128, 2, TB, 2, 128]
        q1, _, q3 = _split16(_stat_qk(wq_c), 16.0)
        k1, _, k3 = _split16(_stat_qk(wk_c), 16.0)
        wqs = np.ascontiguousarray(
            np.stack([q1, q3], axis=2))                  # [4,128,2,TB,2,128]
        wks = np.ascontiguousarray(np.stack([k1, k3], axis=2))

        wv_c = wv[:, heads, :].reshape(E, HC * D)
        wvm = np.empty((2, 128, 3, TB, 2, 256), E4)
        for vh in range(2):
            N = wv_c[:, 256 * vh:256 * vh + 256]
            N = N.reshape(TB, 2, 128, 256).transpose(2, 0, 1, 3)
            s1, s2, s3 = _split16(N, 16.0)
            wvm[vh, :, 0], wvm[vh, :, 1], wvm[vh, :, 2] = s1, s2, s3

        wi_c = wi[:, FC * g:FC * (g + 1)]
        wis = np.empty((FT // 2, 128, 2, 3, TB, 2, 128), E4)
        for ft in range(FT):
            M = wi_c[:, 128 * ft:128 * ft + 128]
            M = M.reshape(TB, 2, 128, 128).transpose(2, 0, 1, 3)
            s1, s2, s3 = _split16(M, 16.0)
            fp, j = divmod(ft, 2)
            wis[fp, :, j, 0], wis[fp, :, j, 1], wis[fp, :, j, 2] = s1, s2, s3

        wmo_c = wmo[FC * g:FC * (g + 1), :]               # [FC, E]
        wm = wmo_c.reshape(FT, 128, E).transpose(1, 0, 2)  # [128, FT, E]
        m1 = _f8(32.0 * wm)
        m2 = _f8(32.0 * wm - m1.astype(np.float32))
        wmm = np.stack([m1, m2])

        wo_c = wo[heads]                                   # [HC, D, E]
        wos = wo_c.reshape(4, 2, 64, E).transpose(0, 1, 2, 3) \
            .reshape(4, 128, E).transpose(1, 0, 2)         # [128, 4, E]
        wos = np.ascontiguousarray(wos).astype(BF)

        in_maps.append(dict(
            x8=np.ascontiguousarray(x8), xr=np.ascontiguousarray(xr),
            wqs=wqs, wks=wks, wvm=wvm, wis=wis, wmm=wmm,
            wos=wos, band=band, bfut=bfut,
            ident=np.eye(128, dtype=np.float32).astype(BF),
        ))
    return in_maps, causal, inputs


def run(trace=False, **kw):
    in_maps, causal, inputs = _prep_in_maps(**kw)
    nc = _get_nc(causal)
    res = run_bass_kernel_spmd(nc, in_maps, list(range(NCORES)), trace=trace)
    out = np.empty((B, L, E), dtype=np.float32)
    for b in range(B):
        out[b] = (inputs[b]
                  + res.results[2 * b]["attn_out"].astype(np.float32)
                  + res.results[2 * b]["mlp_out"].astype(np.float32)
                  + res.results[2 * b + 1]["attn_out"].astype(np.float32)
                  + res.results[2 * b + 1]["mlp_out"].astype(np.float32))
    return out, res


def kernel(**inputs):
    out, _ = run(**inputs)
    return out


# revision 62
# speedup vs baseline: 1.0071x; 1.0071x over previous
"""Trainium2 Bass kernel for nn_DecoderLayer_19851338842283.

8 cores: data-parallel over batch (4) x tensor-parallel (2) over heads/mlp_dim.
fp8(e4m3) DoubleRow matmuls for projections / MLP (with host-side residual
weight passes for accuracy), fp8-DR d-split scores, bf16 exp/ctx/out-proj.
Host sums the two tensor-parallel partials and adds the residual.

Scheduling/structure (402.7us -> 307.4us on the TimelineSim cost model):
- consolidated DMAs (few big transfers, weights interleaved with x quarters
  so the PE starts ~5us in instead of ~45us)
- q/k projections at 2 fp8 passes (s1@x8 + s3@xr)
- rel-pos band applied as exp(s)*exp(b) on the Pool engine (off PE/DVE)
- two k-tiles of scores share a 2-bank psum tile so one exp instruction
  covers both (halves the Act per-instruction overhead)
- ctx computed in [q, d+1] layout (65-col moving operand: 2.3x less PE time
  than the [d, q] orientation), denominator via the ones column; per-head
  qc-major drains keep psum accumulation groups contiguous per bank (HW
  accumulation state is bank-level - interleaved groups corrupt)
- normalized ctx transposed back to [d, q] with PE transposes (2 heads per
  transpose), deferred a few units to stay off the DVE critical path
- producer (scores/exp/band) runs a full head ahead of the ctx drain;
  MLP-in units interleave into attention at a fixed cadence
- bf16 outputs, halved output DMA traffic
"""

import ml_dtypes
import numpy as np

import concourse.bacc as bacc
import concourse.mybir as mybir
import concourse.tile as tile
from concourse.bass_utils import run_bass_kernel_spmd

F32 = mybir.dt.float32
BF16 = mybir.dt.bfloat16
FP8 = mybir.dt.float8e4
Act = mybir.ActivationFunctionType
Alu = mybir.AluOpType
DR = mybir.MatmulPerfMode.DoubleRow
E4 = ml_dtypes.float8_e4m3
BF = ml_dtypes.bfloat16

B, L, E, H, D, F = 4, 2048, 1024, 16, 64, 4096
HC = H // 2          # heads per core = 8
FC = F // 2          # mlp dim per core = 2048
NCORES = 8
ET = E // 128        # 8
LT = L // 128        # 16
FT = FC // 128       # 16
NSUP = L // 512      # 4
TB = E // 256        # 4 DR pair-blocks over E
PB = FC // 256       # 8 DR pair-blocks over FC
BAND_OFF = 128
BAND_W = 384
NUM_BUCKETS = 32
QSC = np.float32(0.125 ** 0.5 / 16.0)   # psum->q8/k8 copy scale


def _build(causal: bool):
    nc = bacc.Bacc("TRN2", target_bir_lowering=False, debug=False,
                   num_devices=NCORES)
    x8_d = nc.dram_tensor("x8", [128, ET, L], FP8, kind="ExternalInput").ap()
    xr_d = nc.dram_tensor("xr", [128, ET, L], FP8, kind="ExternalInput").ap()
    # q/k weights: [tl, part, ps(2: s1|s3), tb, 2, 128]
    wqs_d = nc.dram_tensor("wqs", [4, 128, 2, TB, 2, 128], FP8,
                           kind="ExternalInput").ap()
    wks_d = nc.dram_tensor("wks", [4, 128, 2, TB, 2, 128], FP8,
                           kind="ExternalInput").ap()
    # v weights: [vh, part, ps(3), tb, 2, 256]
    wvm_d = nc.dram_tensor("wvm", [2, 128, 3, TB, 2, 256], FP8,
                           kind="ExternalInput").ap()
    # wi weights: [ftpair, part, j(2), ps(3), tb, 2, 128]
    wis_d = nc.dram_tensor("wis", [FT // 2, 128, 2, 3, TB, 2, 128], FP8,
                           kind="ExternalInput").ap()
    wmm_d = nc.dram_tensor("wmm", [2, 128, FT, E], FP8,
                           kind="ExternalInput").ap()
    wos_d = nc.dram_tensor("wos", [128, 4, E], BF16, kind="ExternalInput").ap()
    band_d = nc.dram_tensor("band", [128, HC, BAND_W], BF16,
                            kind="ExternalInput").ap()
    ident_d = nc.dram_tensor("ident", [128, 128], BF16,
                             kind="ExternalInput").ap()
    bfut_d = nc.dram_tensor("bfut", [128, HC], F32, kind="ExternalInput").ap()
    attn_d = nc.dram_tensor("attn_out", [L, E], BF16,
                            kind="ExternalOutput").ap()
    mlp_d = nc.dram_tensor("mlp_out", [L, E], BF16,
                           kind="ExternalOutput").ap()

    with tile.TileContext(nc) as tc:
        with (
            tc.tile_pool(name="pbig", bufs=1) as pbig,
            tc.tile_pool(name="pqk", bufs=4) as pqk,
            tc.tile_pool(name="pva", bufs=1) as pva,
        ):
            x8 = pbig.tile([128, ET, L], FP8, tag="x8", name="x8")
            wms = [pbig.tile([128, FT, E], FP8, tag=f"wm{ps}", name=f"wm{ps}")
                   for ps in range(2)]
            xr = pbig.tile([128, ET, L], FP8, tag="xr", name="xr")
            h8 = pbig.tile([128, FT, L], FP8, tag="h8", name="h8")
            q8s = [pqk.tile([128, 2, L], FP8, tag="qk", name=f"q8_{g}")
                   for g in range(2)]
            k8s = [pqk.tile([128, 2, L], FP8, tag="qk", name=f"k8_{g}")
                   for g in range(2)]
            va_all = pva.tile([128, LT * HC * 65], BF16, tag="va", name="va")

            # ---------------- q/k/v projections -----------------------------
            with (
                tc.tile_pool(name="pw", bufs=10) as pw,
                tc.tile_pool(name="pps1", bufs=3, space="PSUM") as pps1,
            ):
                # Allocate all stationary-weight tiles, then issue DMAs in an
                # order that lets the PE start as early as possible: first
                # q-weight tile, then x quarters interleaved with the
                # remaining weight tiles.
                sts_q = [pw.tile([128, 2, TB, 2, 128], FP8, tag="w",
                                 name=f"stq{tl}") for tl in range(4)]
                sts_k = [pw.tile([128, 2, TB, 2, 128], FP8, tag="w",
                                 name=f"stk{tl}") for tl in range(4)]
                wvs = [pw.tile([128, 3, TB, 2, 256], FP8, tag="w",
                               name=f"wv{vh}") for vh in range(2)]

                def xq(c):
                    nc.sync.dma_start(x8[:, :, 512 * c:512 * c + 512],
                                      x8_d[:, :, 512 * c:512 * c + 512])
                    nc.sync.dma_start(xr[:, :, 512 * c:512 * c + 512],
                                      xr_d[:, :, 512 * c:512 * c + 512])

                nc.sync.dma_start(sts_q[0][:], wqs_d[0])
                xq(0)
                nc.sync.dma_start(sts_k[0][:], wks_d[0])
                for tl in range(1, 4):
                    nc.sync.dma_start(sts_q[tl][:], wqs_d[tl])
                    nc.sync.dma_start(sts_k[tl][:], wks_d[tl])
                for c in range(1, NSUP):
                    xq(c)
                for vh in range(2):
                    nc.sync.dma_start(wvs[vh][:], wvm_d[vh])
                for ps in range(2):
                    nc.sync.dma_start(wms[ps][:], wmm_d[ps])

                # q/k: 2 passes (s1 @ x8 + s3 @ xr)
                def xmov2(ps, tb, c0, cw):
                    src = x8 if ps == 0 else xr
                    return src[:, 2 * tb:2 * tb + 2, c0:c0 + cw]

                # v / wi: 3 passes (s1 @ x8 + s2 @ x8 + s3 @ xr)
                def xmov3(ps, tb, c0, cw):
                    src = x8 if ps < 2 else xr
                    return src[:, 2 * tb:2 * tb + 2, c0:c0 + cw]

                # c-major so the PE consumes x quarters in DMA-arrival order;
                # q/k alternated to match the weight-DMA issue order
                for c in range(NSUP):
                    for tl in range(4):          # (g, dpair)
                        for sts, dsts in ((sts_q, q8s), (sts_k, k8s)):
                            g, dp = divmod(tl, 2)
                            st = sts[tl]
                            acc = pps1.tile([128, 512], F32, tag="ps1")
                            for ps in range(2):
                                for tb in range(TB):
                                    nc.tensor.matmul(
                                        acc[:], st[:, ps, tb],
                                        xmov2(ps, tb, 512 * c, 512),
                                        start=(ps == 0 and tb == 0),
                                        stop=(ps == 1 and tb == TB - 1),
                                        perf_mode=DR)
                            nc.scalar.mul(
                                dsts[g][:, dp, 512 * c:512 * c + 512],
                                acc[:], float(QSC))

                # ---------------- v projection (bf16 va + ones) -------------
                ones_c = nc.const_aps.tensor(1.0, [128, HC, 1], BF16)
                for vh in range(2):
                    wv = wvs[vh]
                    for lt in range(LT):
                        acc = pps1.tile([128, 256], F32, tag="ps1")
                        for ps in range(3):
                            for tb in range(TB):
                                nc.tensor.matmul(
                                    acc[:], xmov3(ps, tb, 128 * lt, 128),
                                    wv[:, ps, tb],
                                    start=(ps == 0 and tb == 0),
                                    stop=(ps == 2 and tb == TB - 1),
                                    perf_mode=DR)
                        va3 = va_all[:, 520 * lt:520 * lt + 520] \
                            .rearrange("p (h c) -> p h c", h=HC)
                        nc.vector.tensor_scalar_mul(
                            va3[:, 4 * vh:4 * vh + 4, 0:64],
                            acc[:].rearrange("p (h c) -> p h c", h=4),
                            1.0 / 16.0)
                        if vh == 0:
                            nc.vector.tensor_copy(va3[:, :, 64:65], ones_c)

            # -------- attention (+ interleaved MLP-in) ----------------------
            with (
                tc.tile_pool(name="pwi", bufs=3) as pwi,
                tc.tile_pool(name="pband", bufs=1) as pband,
                tc.tile_pool(name="pct", bufs=12) as pct,
                tc.tile_pool(name="pwo", bufs=1) as pwo,
                tc.tile_pool(name="pexp", bufs=13) as pexp,
                tc.tile_pool(name="prr", bufs=2) as prr,
                tc.tile_pool(name="pcsb", bufs=3) as pcsb,
                tc.tile_pool(name="poba", bufs=2) as poba,
                tc.tile_pool(name="pps", bufs=2, space="PSUM") as pps,
                tc.tile_pool(name="pctx", bufs=1, space="PSUM") as pctx,
                tc.tile_pool(name="pasm", bufs=1, space="PSUM") as pasm,
                tc.tile_pool(name="pout", bufs=1, space="PSUM") as pout,
                tc.tile_pool(name="pmps", bufs=1, space="PSUM") as pmps,
            ):
                band_sb = pband.tile([128, HC * BAND_W], BF16, tag="band")
                band3 = band_sb[:].rearrange("p (h w) -> p h w", h=HC)
                nc.sync.dma_start(band3, band_d[:, :, :])
                ident = pband.tile([128, 128], BF16, tag="ident")
                nc.sync.dma_start(ident[:], ident_d)
                bfut_sb = pband.tile([128, HC], F32, tag="bfut")
                if not causal:
                    nc.sync.dma_start(bfut_sb[:], bfut_d)
                wos = pwo.tile([128, 4, E], BF16, tag="wo", name="wos")
                nc.sync.dma_start(wos[:], wos_d)

                mlp_units = [(ft, c) for ft in range(FT) for c in range(NSUP)]
                mo_units = [(lt, ec) for lt in range(LT) for ec in range(2)]
                h4a = h8[:].rearrange("p (j pb) l -> p j pb l", j=2)
                wm4a = [w[:].rearrange("p (j pb) e -> p j pb e", j=2)
                        for w in wms]

                mlp_i = 0
                wi_sts = {}

                def wi_fetch(fp):
                    if fp >= FT // 2 or fp in wi_sts:
                        return
                    t = pwi.tile([128, 2, 3, TB, 2, 128], FP8, tag="wi",
                                 name=f"wist{fp}")
                    nc.sync.dma_start(t[:], wis_d[fp])
                    wi_sts[fp] = t

                wi_fetch(0)
                wi_fetch(1)
                wi_fetch(2)

                def emit_mlp_unit():
                    nonlocal mlp_i
                    if mlp_i >= len(mlp_units):
                        return
                    ft, c = mlp_units[mlp_i]
                    mlp_i += 1
                    if c == 0:
                        wi_fetch(ft // 2 + 2)
                    t = wi_sts[ft // 2]
                    acc = pmps.tile([128, 512], F32, tag="mps")
                    for ps in range(3):
                        for tb in range(TB):
                            nc.tensor.matmul(
                                acc[:], t[:, ft % 2, ps, tb],
                                xmov3(ps, tb, 512 * c, 512),
                                start=(ps == 0 and tb == 0),
                                stop=(ps == 2 and tb == TB - 1),
                                perf_mode=DR)
                    nc.vector.tensor_scalar(
                        out=h8[:, ft, 512 * c:512 * c + 512], in0=acc[:],
                        scalar1=1.0 / 16.0, scalar2=0.0,
                        op0=Alu.mult, op1=Alu.max)

                ob_tiles = {}

                def outproj(s_prev, cts_prev, chunks, last=False):
                    for qt, ec in chunks:
                        acc = pout.tile([128, 512], F32, tag="out",
                                        name="opacc")
                        for p in range(4):
                            nc.tensor.matmul(
                                acc[:],
                                cts_prev[p][:, 128 * qt:128 * qt + 128],
                                wos[:, p, 512 * ec:512 * ec + 512],
                                start=(p == 0), stop=(p == 3))
                        if ec == 0:
                            ob_tiles[qt] = poba.tile([128, 1024], BF16,
                                                     tag="ob",
                                                     name=f"ob{s_prev}_{qt}")
                        ob = ob_tiles[qt]
                        if last and ec == 1 and qt == 3:
                            nc.scalar.copy(ob[:, 512 * ec:512 * ec + 512],
                                           acc[:])
                        else:
                            nc.vector.tensor_copy(
                                ob[:, 512 * ec:512 * ec + 512], acc[:])
                        if ec == 1:
                            qs0 = 512 * s_prev
                            nc.sync.dma_start(
                                attn_d[qs0 + 128 * qt:qs0 + 128 * qt + 128,
                                       :], ob[:])

                def nkt(s):
                    return 4 * (s + 1) if causal else LT

                cts_map = {}
                csb_map = {}
                ea_map = {}
                ktc = [0]

                pending = []     # deferred transpose+copy blocks

                def normalize(s, cps, h):
                    # cps [128, 4, 65]: per-chunk scale by 1/denominator
                    p = h // 2
                    if h % 2 == 0:
                        csb_map[(s, p)] = pcsb.tile([128, 4, 2, 64], BF16,
                                                    tag="csb",
                                                    name=f"csb{s}_{p}")
                    csb = csb_map[(s, p)]
                    rr = prr.tile([128, 4, 1], F32, tag="rr", name="rr")
                    nc.vector.reciprocal(rr[:], cps[:, :, 64:65])
                    for qc in range(4):
                        nc.vector.tensor_scalar_mul(
                            csb[:, qc, h % 2, :], cps[:, qc, 0:64],
                            rr[:, qc])
                    if h % 2 == 1:
                        # both heads of p normalized: defer the PE transposes
                        # so they don't stall on the DVE normalize above
                        pending.append((s, p, csb, cts_map[s]))
                        del csb_map[(s, p)]

                def finish_pair(force=False):
                    # depth-1 deferral only: outproj chunks read all four
                    # cts[p] tiles, so the last pair's transposes must land
                    # before the next supertile's first outproj chunk
                    if not pending:
                        return
                    s, p, csb, cts = pending.pop(0)
                    asm = pasm.tile([128, 512], BF16, tag="asm",
                                    name=f"asm{s}_{p}")
                    for qc in range(4):
                        nc.tensor.transpose(
                            asm[:, 128 * qc:128 * qc + 128],
                            csb[:, qc, :, :].rearrange("p a b -> p (a b)"),
                            ident[:])
                    nc.vector.tensor_copy(cts[p][:], asm[:])

                def produce_pair(s, h, kt0):
                    # two k-tiles (kt0, kt0+1) share one 2-bank psum tile so
                    # a single exp instruction covers both (halving the Act
                    # per-instruction overhead); the diagonal pairs keep two
                    # exps to skip the masked region
                    qs = 512 * s
                    g, u = divmod(h, 4)
                    psA = pps.tile([128, 2, 512], F32, tag="ps", name="sc")
                    ea = pexp.tile([128, 2, 512], BF16, tag="exp", name="ea")
                    offs = []
                    for j in range(2):
                        kt = kt0 + j
                        k0 = 128 * kt
                        off = min(max(0, k0 - qs), 384) if causal else 0
                        offs.append(off)
                        nc.tensor.matmul(
                            psA[:, j, off:512],
                            k8s[g][32 * u:32 * u + 32, :, k0:k0 + 128],
                            q8s[g][32 * u:32 * u + 32, :, qs + off:qs + 512],
                            start=True, stop=True,
                            perf_mode=DR, tile_position=(32 * u, 0))
                        ul = (min(max(k0 - BAND_OFF - qs, 0), 512)
                              if not causal else 0)
                        if ul > 0:
                            nc.vector.tensor_scalar_add(
                                psA[:, j, 0:ul], psA[:, j, 0:ul],
                                bfut_sb[:, h:h + 1])
                    psF = psA[:].rearrange("p a b -> p (a b)")
                    eaF = ea[:].rearrange("p a b -> p (a b)")
                    if offs[1] == 0:
                        nc.scalar.activation(eaF[:, 0:1024], psF[:, 0:1024],
                                             Act.Exp)
                    else:
                        nc.scalar.activation(eaF[:, offs[0]:512],
                                             psF[:, offs[0]:512], Act.Exp)
                        nc.scalar.activation(eaF[:, 512 + offs[1]:1024],
                                             psF[:, 512 + offs[1]:1024],
                                             Act.Exp)
                    for j in range(2):
                        kt = kt0 + j
                        k0 = 128 * kt
                        off = offs[j]
                        o_lo = max(k0 - BAND_OFF, qs + off)
                        o_hi = min(k0 + 256, qs + 512)
                        if o_hi > o_lo:
                            # exp(s+b) == exp(s)*exp(b): banded rel-pos bias
                            # (and causal-mask zeros) as a Pool multiply; the
                            # head's last pair goes on DVE (lower latency)
                            # since the drain waits on it soonest
                            psl = slice(o_lo - qs, o_hi - qs)
                            bsl = slice(o_lo - (k0 - BAND_OFF),
                                        o_hi - (k0 - BAND_OFF))
                            eng = (nc.vector if kt0 + 2 >= nkt(s)
                                   else nc.gpsimd)
                            eng.tensor_tensor(
                                ea[:, j, psl], ea[:, j, psl],
                                band3[:, h, bsl], Alu.mult)
                        ea_map[(s, h, kt)] = (ea[:, j], off)
                        ktc[0] += 1
                        if ktc[0] % 5 == 0:
                            emit_mlp_unit()

                def consume_head(s, h):
                    # drain the whole head qc-major: each query-chunk's psum
                    # accumulation group is contiguous (HW accumulate state
                    # is per-bank, so groups must not interleave)
                    cps = pctx.tile([128, 4, 65], F32, tag="ctx",
                                    name=f"cps{s}_{h}")
                    kmax = nkt(s) - 1
                    for qc in range(4):
                        lastk = min(kmax, 4 * s + qc) if causal else kmax
                        for kt in range(lastk + 1):
                            ea, off = ea_map[(s, h, kt)]
                            nc.tensor.matmul(
                                cps[:, qc, :],
                                ea[:, 128 * qc:128 * qc + 128],
                                va_all[:, 520 * kt + 65 * h:
                                       520 * kt + 65 * h + 65],
                                start=(kt == 0), stop=(kt == lastk))
                    for kt in range(nkt(s)):
                        ea_map.pop((s, h, kt))
                    normalize(s, cps, h)
                    if s > 0:
                        outproj(s - 1, cts_map[s - 1], [divmod(h, 2)])
                        if h == HC - 1:
                            del cts_map[s - 1]

                heads = [(s, h) for s in range(NSUP) for h in range(HC)]
                for j, (s, h) in enumerate(heads):
                    if h == 0:
                        cts_map[s] = [pct.tile([128, 512], BF16, tag="ct",
                                               name=f"ct{s}_{p}")
                                      for p in range(4)]
                    drain_at = 4 if nkt(s) >= 12 else 2
                    for kt0 in range(0, nkt(s), 2):
                        produce_pair(s, h, kt0)
                        if kt0 == 2:
                            finish_pair()
                        if kt0 == drain_at and j >= 1:
                            consume_head(*heads[j - 1])
                consume_head(*heads[-1])
                while pending:
                    finish_pair(force=True)
                outproj(NSUP - 1, cts_map[NSUP - 1],
                        [(qt, ec) for qt in range(4) for ec in range(2)],
                        last=True)
                while mlp_i < len(mlp_units):
                    emit_mlp_unit()

            # ---------------- MLP down-projection ---------------------------
            with (
                tc.tile_pool(name="pobb", bufs=2) as pobb,
                tc.tile_pool(name="pps3", bufs=3, space="PSUM") as pps3,
            ):
                h4 = h8[:].rearrange("p (j pb) l -> p j pb l", j=2)
                wm4 = [w[:].rearrange("p (j pb) e -> p j pb e", j=2)
                       for w in wms]
                for lt in range(LT):
                    ob = pobb.tile([128, 1024], BF16, tag="ob")
                    for ec in range(2):
                        acc = pps3.tile([128, 512], F32, tag="ps3")
                        for ps in range(2):
                            for pb in range(PB):
                                nc.tensor.matmul(
                                    acc[:],
                                    h4[:, :, pb, 128 * lt:128 * lt + 128],
                                    wm4[ps][:, :, pb,
                                            512 * ec:512 * ec + 512],
                                    start=(ps == 0 and pb == 0),
                                    stop=(ps == 1 and pb == PB - 1),
                                    perf_mode=DR)
                        nc.scalar.mul(ob[:, 512 * ec:512 * ec + 512],
                                      acc[:], 1.0 / 32.0)
                        nc.sync.dma_start(
                            mlp_d[128 * lt:128 * lt + 128,
                                  512 * ec:512 * ec + 512],
                            ob[:, 512 * ec:512 * ec + 512])

    nc.compile()
    return nc


_NC_CACHE = {}


def _get_nc(causal: bool):
    if causal not in _NC_CACHE:
        _NC_CACHE[causal] = _build(causal)
    return _NC_CACHE[causal]


def _bucket(n):
    n = np.asarray(n)
    nf = np.maximum(n.astype(np.float32), np.float32(1.0))
    v = np.log(nf / np.float32(16.0)).astype(np.float32)
    v = (v / np.float32(np.log(8.0))) * np.float32(16.0)
    val_large = np.minimum(16 + v.astype(np.int32), NUM_BUCKETS - 1)
    return np.where(n < 16, n, val_large)


def _make_band(rel_emb, heads, causal):
    """exp() of the banded rel-pos bias (causal-masked entries -> 0)."""
    d = np.arange(-(BAND_OFF + 127), 256)
    pos = np.maximum(d, 0)
    bv = rel_emb[_bucket(pos)][:, heads] - rel_emb[NUM_BUCKETS - 1][heads]
    bv = np.where(d[:, None] >= 113, np.float32(0.0), bv)
    bv = np.exp(bv).astype(np.float32)
    if causal:
        bv = np.where(d[:, None] < 0, np.float32(0.0), bv)
    else:
        fut = np.exp(rel_emb[0][heads] - rel_emb[NUM_BUCKETS - 1][heads])
        bv = np.where(d[:, None] < 0, fut[None, :], bv)
    i = np.arange(128)[:, None]
    j = np.arange(BAND_W)[None, :]
    idx = (j - BAND_OFF - i) + (BAND_OFF + 127)
    return bv.astype(np.float32)[idx]          # [128, BAND_W, HC]


def _f8(a):
    return np.ascontiguousarray(a, dtype=np.float32).astype(E4)


def _split16(w, s):
    """-> (e4m3(s*w), e4m3(s*w - f32(e4m3(s*w))), e4m3(f32(e4m3(s*w))/s))"""
    w = np.asarray(w, np.float32)
    s1 = _f8(s * w)
    f1 = s1.astype(np.float32)
    s2 = _f8(s * w - f1)
    s3 = _f8(f1 / s)
    return s1, s2, s3


def _stat_qk(w_c):
    """w_c [E, HC, D] -> [4(tile), 128, TB, 2, 128] in f32 (pre-quant)."""
    arr = w_c.reshape(E, 2, 4, 2, 32)           # e, g, u, dp, dm
    out = np.empty((4, TB, 128, 2, 128), np.float32)
    for tl in range(4):
        g, dp = divmod(tl, 2)
        M = arr[:, g, :, dp, :].reshape(E, 128)  # m = 32u + dm
        out[tl] = M.reshape(TB, 2, 128, 128).transpose(0, 2, 1, 3)
    return out.transpose(0, 2, 1, 3, 4)          # [4, 128, TB, 2, 128]


def _prep_in_maps(inputs, wq, wk, wv, wo, wi, wmo, rel_emb, decoder_mask):
    inputs = np.asarray(inputs, dtype=np.float32)
    wq = np.asarray(wq, dtype=np.float32)
    wk = np.asarray(wk, dtype=np.float32)
    wv = np.asarray(wv, dtype=np.float32)
    wo = np.asarray(wo, dtype=np.float32)
    wi = np.asarray(wi, dtype=np.float32)
    wmo = np.asarray(wmo, dtype=np.float32)
    rel_emb = np.asarray(rel_emb, dtype=np.float32)
    mask = np.asarray(decoder_mask).reshape(L, L)

    tril = np.tril(np.ones((L, L), dtype=bool))
    if np.array_equal(mask, tril):
        causal = True
    elif mask.all():
        causal = False
    else:
        raise NotImplementedError("only causal or all-true masks supported")

    in_maps = []
    for c in range(NCORES):
        b, g = divmod(c, 2)
        heads = np.arange(HC * g, HC * (g + 1))
        band = _make_band(rel_emb, heads, causal)        # [128, W, HC]
        band = np.ascontiguousarray(band.transpose(0, 2, 1)).astype(BF)
        bfut = np.broadcast_to(
            (rel_emb[0][heads] - rel_emb[NUM_BUCKETS - 1][heads])
            .astype(np.float32), (128, HC)).copy()

        xT = inputs[b].T                                  # [E, L]
        x8 = _f8(xT)
        xr = _f8(16.0 * (xT - x8.astype(np.float32)))
        x8 = x8.reshape(ET, 128, L).transpose(1, 0, 2)    # [128, ET, L]
        xr = xr.reshape(ET, 128, L).transpose(1, 0, 2)

        wq_c = wq[:, heads, :]
        wk_c = wk[:, heads, :]
        # 2-pass q/k: keep only (s1, s3) -> [4, 128, 2, TB, 2, 128]
        q1, _, q3 = _split16(_stat_qk(wq_c), 16.0)
        k1, _, k3 = _split16(_stat_qk(wk_c), 16.0)
        wqs = np.ascontiguousarray(
            np.stack([q1, q3], axis=2))                  # [4,128,2,TB,2,128]
        wks = np.ascontiguousarray(np.stack([k1, k3], axis=2))

        wv_c = wv[:, heads, :].reshape(E, HC * D)
        wvm = np.empty((2, 128, 3, TB, 2, 256), E4)
        for vh in range(2):
            N = wv_c[:, 256 * vh:256 * vh + 256]
            N = N.reshape(TB, 2, 128, 256).transpose(2, 0, 1, 3)
            s1, s2, s3 = _split16(N, 16.0)
            wvm[vh, :, 0], wvm[vh, :, 1], wvm[vh, :, 2] = s1, s2, s3

        wi_c = wi[:, FC * g:FC * (g + 1)]
        wis = np.empty((FT // 2, 128, 2, 3, TB, 2, 128), E4)
        for ft in range(FT):
            M = wi_c[:, 128 * ft:128 * ft + 128]
            M = M.reshape(TB, 2, 128, 128).transpose(2, 0, 1, 3)
            s1, s2, s3 = _split16(M, 16.0)
            fp, j = divmod(ft, 2)
            wis[fp, :, j, 0], wis[fp, :, j, 1], wis[fp, :, j, 2] = s1, s2, s3

        wmo_c = wmo[FC * g:FC * (g + 1), :]               # [FC, E]
        wm = wmo_c.reshape(FT, 128, E).transpose(1, 0, 2)  # [128, FT, E]
        m1 = _f8(32.0 * wm)
        m2 = _f8(32.0 * wm - m1.astype(np.float32))
        wmm = np.stack([m1, m2])

        wo_c = wo[heads]                                   # [HC, D, E]
        wos = wo_c.reshape(4, 2, 64, E).transpose(0, 1, 2, 3) \
            .reshape(4, 128, E).transpose(1, 0, 2)         # [128, 4, E]
        wos = np.ascontiguousarray(wos).astype(BF)

        in_maps.append(dict(
            x8=np.ascontiguousarray(x8), xr=np.ascontiguousarray(xr),
            wqs=wqs, wks=wks, wvm=wvm, wis=wis, wmm=wmm,
            wos=wos, band=band, bfut=bfut,
            ident=np.eye(128, dtype=np.float32).astype(BF),
        ))
    return in_maps, causal, inputs


def run(trace=False, **kw):
    in_maps, causal, inputs = _prep_in_maps(**kw)
    nc = _get_nc(causal)
    res = run_bass_kernel_spmd(nc, in_maps, list(range(NCORES)), trace=trace)
    out = np.empty((B, L, E), dtype=np.float32)
    for b in range(B):
        out[b] = (inputs[b]
                  + res.results[2 * b]["attn_out"].astype(np.float32)
                  + res.results[2 * b]["mlp_out"].astype(np.float32)
                  + res.results[2 * b + 1]["attn_out"].astype(np.float32)
                  + res.results[2 * b + 1]["mlp_out"].astype(np.float32))
    return out, res


def kernel(**inputs):
    out, _ = run(**inputs)
    return out


# revision 63
# speedup vs baseline: 1.0110x; 1.0039x over previous
"""Trainium2 Bass kernel for nn_DecoderLayer_19851338842283.

8 cores: data-parallel over batch (4) x tensor-parallel (2) over heads/mlp_dim.
fp8(e4m3) DoubleRow matmuls for projections / MLP (with host-side residual
weight passes for accuracy), fp8-DR d-split scores, bf16 exp/ctx/out-proj.
Host sums the two tensor-parallel partials and adds the residual.

Scheduling/structure (402.7us -> 307.4us on the TimelineSim cost model):
- consolidated DMAs (few big transfers, weights interleaved with x quarters
  so the PE starts ~5us in instead of ~45us)
- q/k projections at 2 fp8 passes (s1@x8 + s3@xr)
- rel-pos band applied as exp(s)*exp(b) on the Pool engine (off PE/DVE)
- two k-tiles of scores share a 2-bank psum tile so one exp instruction
  covers both (halves the Act per-instruction overhead)
- ctx computed in [q, d+1] layout (65-col moving operand: 2.3x less PE time
  than the [d, q] orientation), denominator via the ones column; per-head
  qc-major drains keep psum accumulation groups contiguous per bank (HW
  accumulation state is bank-level - interleaved groups corrupt)
- normalized ctx transposed back to [d, q] with PE transposes (2 heads per
  transpose), deferred a few units to stay off the DVE critical path
- producer (scores/exp/band) runs a full head ahead of the ctx drain;
  MLP-in units interleave into attention at a fixed cadence
- bf16 outputs, halved output DMA traffic
"""

import ml_dtypes
import numpy as np

import concourse.bacc as bacc
import concourse.mybir as mybir
import concourse.tile as tile
from concourse.bass_utils import run_bass_kernel_spmd

F32 = mybir.dt.float32
BF16 = mybir.dt.bfloat16
FP8 = mybir.dt.float8e4
Act = mybir.ActivationFunctionType
Alu = mybir.AluOpType
DR = mybir.MatmulPerfMode.DoubleRow
E4 = ml_dtypes.float8_e4m3
BF = ml_dtypes.bfloat16

B, L, E, H, D, F = 4, 2048, 1024, 16, 64, 4096
HC = H // 2          # heads per core = 8
FC = F // 2          # mlp dim per core = 2048
NCORES = 8
ET = E // 128        # 8
LT = L // 128        # 16
FT = FC // 128       # 16
NSUP = L // 512      # 4
TB = E // 256        # 4 DR pair-blocks over E
PB = FC // 256       # 8 DR pair-blocks over FC
BAND_OFF = 128
BAND_W = 384
NUM_BUCKETS = 32
QSC = np.float32(0.125 ** 0.5 / 16.0)   # psum->q8/k8 copy scale


def _build(causal: bool):
    nc = bacc.Bacc("TRN2", target_bir_lowering=False, debug=False,
                   num_devices=NCORES)
    x8_d = nc.dram_tensor("x8", [128, ET, L], FP8, kind="ExternalInput").ap()
    xr_d = nc.dram_tensor("xr", [128, ET, L], FP8, kind="ExternalInput").ap()
    # q/k weights: [tl, part, ps(2: s1|s3), tb, 2, 128]
    wqs_d = nc.dram_tensor("wqs", [4, 128, 2, TB, 2, 128], FP8,
                           kind="ExternalInput").ap()
    wks_d = nc.dram_tensor("wks", [4, 128, 2, TB, 2, 128], FP8,
                           kind="ExternalInput").ap()
    # v weights: [vh, part, ps(3), tb, 2, 256]
    wvm_d = nc.dram_tensor("wvm", [2, 128, 3, TB, 2, 256], FP8,
                           kind="ExternalInput").ap()
    # wi weights: [ftpair, part, j(2), ps(3), tb, 2, 128]
    wis_d = nc.dram_tensor("wis", [FT // 2, 128, 2, 3, TB, 2, 128], FP8,
                           kind="ExternalInput").ap()
    wmm_d = nc.dram_tensor("wmm", [2, 128, FT, E], FP8,
                           kind="ExternalInput").ap()
    wos_d = nc.dram_tensor("wos", [128, 4, E], BF16, kind="ExternalInput").ap()
    band_d = nc.dram_tensor("band", [128, HC, BAND_W], BF16,
                            kind="ExternalInput").ap()
    ident_d = nc.dram_tensor("ident", [128, 128], BF16,
                             kind="ExternalInput").ap()
    bfut_d = nc.dram_tensor("bfut", [128, HC], F32, kind="ExternalInput").ap()
    attn_d = nc.dram_tensor("attn_out", [L, E], BF16,
                            kind="ExternalOutput").ap()
    mlp_d = nc.dram_tensor("mlp_out", [L, E], BF16,
                           kind="ExternalOutput").ap()

    with tile.TileContext(nc) as tc:
        with (
            tc.tile_pool(name="pbig", bufs=1) as pbig,
            tc.tile_pool(name="pqk", bufs=4) as pqk,
            tc.tile_pool(name="pva", bufs=1) as pva,
        ):
            x8 = pbig.tile([128, ET, L], FP8, tag="x8", name="x8")
            wms = [pbig.tile([128, FT, E], FP8, tag=f"wm{ps}", name=f"wm{ps}")
                   for ps in range(2)]
            xr = pbig.tile([128, ET, L], FP8, tag="xr", name="xr")
            h8 = pbig.tile([128, FT, L], FP8, tag="h8", name="h8")
            q8s = [pqk.tile([128, 2, L], FP8, tag="qk", name=f"q8_{g}")
                   for g in range(2)]
            k8s = [pqk.tile([128, 2, L], FP8, tag="qk", name=f"k8_{g}")
                   for g in range(2)]
            va_all = pva.tile([128, LT * HC * 65], BF16, tag="va", name="va")

            # ---------------- q/k/v projections -----------------------------
            with (
                tc.tile_pool(name="pw", bufs=10) as pw,
                tc.tile_pool(name="pps1", bufs=3, space="PSUM") as pps1,
            ):
                # Allocate all stationary-weight tiles, then issue DMAs in an
                # order that lets the PE start as early as possible: first
                # q-weight tile, then x quarters interleaved with the
                # remaining weight tiles.
                sts_q = [pw.tile([128, 2, TB, 2, 128], FP8, tag="w",
                                 name=f"stq{tl}") for tl in range(4)]
                sts_k = [pw.tile([128, 2, TB, 2, 128], FP8, tag="w",
                                 name=f"stk{tl}") for tl in range(4)]
                wvs = [pw.tile([128, 3, TB, 2, 256], FP8, tag="w",
                               name=f"wv{vh}") for vh in range(2)]

                def xq(c):
                    nc.sync.dma_start(x8[:, :, 512 * c:512 * c + 512],
                                      x8_d[:, :, 512 * c:512 * c + 512])
                    nc.sync.dma_start(xr[:, :, 512 * c:512 * c + 512],
                                      xr_d[:, :, 512 * c:512 * c + 512])

                nc.sync.dma_start(sts_q[0][:], wqs_d[0])
                xq(0)
                nc.sync.dma_start(sts_k[0][:], wks_d[0])
                for tl in range(1, 4):
                    nc.sync.dma_start(sts_q[tl][:], wqs_d[tl])
                    nc.sync.dma_start(sts_k[tl][:], wks_d[tl])
                for c in range(1, NSUP):
                    xq(c)
                for vh in range(2):
                    nc.sync.dma_start(wvs[vh][:], wvm_d[vh])
                for ps in range(2):
                    nc.sync.dma_start(wms[ps][:], wmm_d[ps])

                # q/k: 2 passes (s1 @ x8 + s3 @ xr)
                def xmov2(ps, tb, c0, cw):
                    src = x8 if ps == 0 else xr
                    return src[:, 2 * tb:2 * tb + 2, c0:c0 + cw]

                # v / wi: 3 passes (s1 @ x8 + s2 @ x8 + s3 @ xr)
                def xmov3(ps, tb, c0, cw):
                    src = x8 if ps < 2 else xr
                    return src[:, 2 * tb:2 * tb + 2, c0:c0 + cw]

                # c-major so the PE consumes x quarters in DMA-arrival order;
                # q/k alternated to match the weight-DMA issue order
                for c in range(NSUP):
                    for tl in range(4):          # (g, dpair)
                        for sts, dsts in ((sts_q, q8s), (sts_k, k8s)):
                            g, dp = divmod(tl, 2)
                            st = sts[tl]
                            acc = pps1.tile([128, 512], F32, tag="ps1")
                            for ps in range(2):
                                for tb in range(TB):
                                    nc.tensor.matmul(
                                        acc[:], st[:, ps, tb],
                                        xmov2(ps, tb, 512 * c, 512),
                                        start=(ps == 0 and tb == 0),
                                        stop=(ps == 1 and tb == TB - 1),
                                        perf_mode=DR)
                            nc.scalar.mul(
                                dsts[g][:, dp, 512 * c:512 * c + 512],
                                acc[:], float(QSC))

                # ---------------- v projection (bf16 va + ones) -------------
                ones_c = nc.const_aps.tensor(1.0, [128, HC, 1], BF16)
                for vh in range(2):
                    wv = wvs[vh]
                    for lt in range(LT):
                        acc = pps1.tile([128, 256], F32, tag="ps1")
                        for ps in range(3):
                            for tb in range(TB):
                                nc.tensor.matmul(
                                    acc[:], xmov3(ps, tb, 128 * lt, 128),
                                    wv[:, ps, tb],
                                    start=(ps == 0 and tb == 0),
                                    stop=(ps == 2 and tb == TB - 1),
                                    perf_mode=DR)
                        va3 = va_all[:, 520 * lt:520 * lt + 520] \
                            .rearrange("p (h c) -> p h c", h=HC)
                        nc.vector.tensor_scalar_mul(
                            va3[:, 4 * vh:4 * vh + 4, 0:64],
                            acc[:].rearrange("p (h c) -> p h c", h=4),
                            1.0 / 16.0)
                        if vh == 0:
                            nc.vector.tensor_copy(va3[:, :, 64:65], ones_c)

            # -------- attention (+ interleaved MLP-in) ----------------------
            with (
                tc.tile_pool(name="pwi", bufs=3) as pwi,
                tc.tile_pool(name="pband", bufs=1) as pband,
                tc.tile_pool(name="pct", bufs=12) as pct,
                tc.tile_pool(name="pwo", bufs=1) as pwo,
                tc.tile_pool(name="pexp", bufs=13) as pexp,
                tc.tile_pool(name="prr", bufs=2) as prr,
                tc.tile_pool(name="pcsb", bufs=3) as pcsb,
                tc.tile_pool(name="poba", bufs=2) as poba,
                tc.tile_pool(name="pps", bufs=2, space="PSUM") as pps,
                tc.tile_pool(name="pctx", bufs=1, space="PSUM") as pctx,
                tc.tile_pool(name="pasm", bufs=1, space="PSUM") as pasm,
                tc.tile_pool(name="pout", bufs=1, space="PSUM") as pout,
                tc.tile_pool(name="pmps", bufs=1, space="PSUM") as pmps,
            ):
                band_sb = pband.tile([128, HC * BAND_W], BF16, tag="band")
                band3 = band_sb[:].rearrange("p (h w) -> p h w", h=HC)
                nc.sync.dma_start(band3, band_d[:, :, :])
                ident = pband.tile([128, 128], BF16, tag="ident")
                nc.sync.dma_start(ident[:], ident_d)
                bfut_sb = pband.tile([128, HC], F32, tag="bfut")
                if not causal:
                    nc.sync.dma_start(bfut_sb[:], bfut_d)
                wos = pwo.tile([128, 4, E], BF16, tag="wo", name="wos")
                nc.sync.dma_start(wos[:], wos_d)

                mlp_units = [(ft, c) for ft in range(FT) for c in range(NSUP)]
                mo_units = [(lt, ec) for lt in range(LT) for ec in range(2)]
                h4a = h8[:].rearrange("p (j pb) l -> p j pb l", j=2)
                wm4a = [w[:].rearrange("p (j pb) e -> p j pb e", j=2)
                        for w in wms]

                mlp_i = 0
                wi_sts = {}

                def wi_fetch(fp):
                    if fp >= FT // 2 or fp in wi_sts:
                        return
                    t = pwi.tile([128, 2, 3, TB, 2, 128], FP8, tag="wi",
                                 name=f"wist{fp}")
                    nc.sync.dma_start(t[:], wis_d[fp])
                    wi_sts[fp] = t

                wi_fetch(0)
                wi_fetch(1)
                wi_fetch(2)

                def emit_mlp_unit():
                    nonlocal mlp_i
                    if mlp_i >= len(mlp_units):
                        return
                    ft, c = mlp_units[mlp_i]
                    mlp_i += 1
                    if c == 0:
                        wi_fetch(ft // 2 + 2)
                    t = wi_sts[ft // 2]
                    acc = pmps.tile([128, 512], F32, tag="mps")
                    for ps in range(3):
                        for tb in range(TB):
                            nc.tensor.matmul(
                                acc[:], t[:, ft % 2, ps, tb],
                                xmov3(ps, tb, 512 * c, 512),
                                start=(ps == 0 and tb == 0),
                                stop=(ps == 2 and tb == TB - 1),
                                perf_mode=DR)
                    nc.vector.tensor_scalar(
                        out=h8[:, ft, 512 * c:512 * c + 512], in0=acc[:],
                        scalar1=1.0 / 16.0, scalar2=0.0,
                        op0=Alu.mult, op1=Alu.max)

                ob_tiles = {}

                def outproj(s_prev, cts_prev, chunks, last=False):
                    for qt, ec in chunks:
                        acc = pout.tile([128, 512], F32, tag="out",
                                        name="opacc")
                        for p in range(4):
                            nc.tensor.matmul(
                                acc[:],
                                cts_prev[p][:, 128 * qt:128 * qt + 128],
                                wos[:, p, 512 * ec:512 * ec + 512],
                                start=(p == 0), stop=(p == 3))
                        if ec == 0:
                            ob_tiles[qt] = poba.tile([128, 1024], BF16,
                                                     tag="ob",
                                                     name=f"ob{s_prev}_{qt}")
                        ob = ob_tiles[qt]
                        if last and ec == 1 and qt == 3:
                            nc.scalar.copy(ob[:, 512 * ec:512 * ec + 512],
                                           acc[:])
                        else:
                            nc.vector.tensor_copy(
                                ob[:, 512 * ec:512 * ec + 512], acc[:])
                        if ec == 1:
                            qs0 = 512 * s_prev
                            nc.sync.dma_start(
                                attn_d[qs0 + 128 * qt:qs0 + 128 * qt + 128,
                                       :], ob[:])

                def nkt(s):
                    return 4 * (s + 1) if causal else LT

                cts_map = {}
                csb_map = {}
                ea_map = {}
                ktc = [0]

                pending = []     # deferred transpose+copy blocks

                def normalize(s, cps, h):
                    # cps [128, 4, 65]: per-chunk scale by 1/denominator
                    p = h // 2
                    if h % 2 == 0:
                        csb_map[(s, p)] = pcsb.tile([128, 4, 2, 64], BF16,
                                                    tag="csb",
                                                    name=f"csb{s}_{p}")
                    csb = csb_map[(s, p)]
                    rr = prr.tile([128, 4, 1], F32, tag="rr", name="rr")
                    nc.vector.reciprocal(rr[:], cps[:, :, 64:65])
                    for qc in range(4):
                        nc.vector.tensor_scalar_mul(
                            csb[:, qc, h % 2, :], cps[:, qc, 0:64],
                            rr[:, qc])
                    if h % 2 == 1:
                        # both heads of p normalized: defer the PE transposes
                        # so they don't stall on the DVE normalize above
                        pending.append((s, p, csb, cts_map[s]))
                        del csb_map[(s, p)]

                def finish_pair(force=False):
                    # depth-1 deferral only: outproj chunks read all four
                    # cts[p] tiles, so the last pair's transposes must land
                    # before the next supertile's first outproj chunk
                    if not pending:
                        return
                    s, p, csb, cts = pending.pop(0)
                    asm = pasm.tile([128, 512], BF16, tag="asm",
                                    name=f"asm{s}_{p}")
                    for qc in range(4):
                        nc.tensor.transpose(
                            asm[:, 128 * qc:128 * qc + 128],
                            csb[:, qc, :, :].rearrange("p a b -> p (a b)"),
                            ident[:])
                    nc.vector.tensor_copy(cts[p][:], asm[:])

                def produce_pair(s, h, kt0):
                    # two k-tiles (kt0, kt0+1) share one 2-bank psum tile so
                    # a single exp instruction covers both (halving the Act
                    # per-instruction overhead); the diagonal pairs keep two
                    # exps to skip the masked region
                    qs = 512 * s
                    g, u = divmod(h, 4)
                    psA = pps.tile([128, 2, 512], F32, tag="ps", name="sc")
                    ea = pexp.tile([128, 2, 512], BF16, tag="exp", name="ea")
                    offs = []
                    for j in range(2):
                        kt = kt0 + j
                        k0 = 128 * kt
                        off = min(max(0, k0 - qs), 384) if causal else 0
                        offs.append(off)
                        nc.tensor.matmul(
                            psA[:, j, off:512],
                            k8s[g][32 * u:32 * u + 32, :, k0:k0 + 128],
                            q8s[g][32 * u:32 * u + 32, :, qs + off:qs + 512],
                            start=True, stop=True,
                            perf_mode=DR, tile_position=(32 * u, 0))
                        ul = (min(max(k0 - BAND_OFF - qs, 0), 512)
                              if not causal else 0)
                        if ul > 0:
                            nc.vector.tensor_scalar_add(
                                psA[:, j, 0:ul], psA[:, j, 0:ul],
                                bfut_sb[:, h:h + 1])
                    psF = psA[:].rearrange("p a b -> p (a b)")
                    eaF = ea[:].rearrange("p a b -> p (a b)")
                    if offs[1] == 0:
                        nc.scalar.activation(eaF[:, 0:1024], psF[:, 0:1024],
                                             Act.Exp)
                    else:
                        nc.scalar.activation(eaF[:, offs[0]:512],
                                             psF[:, offs[0]:512], Act.Exp)
                        nc.scalar.activation(eaF[:, 512 + offs[1]:1024],
                                             psF[:, 512 + offs[1]:1024],
                                             Act.Exp)
                    for j in range(2):
                        kt = kt0 + j
                        k0 = 128 * kt
                        off = offs[j]
                        o_lo = max(k0 - BAND_OFF, qs + off)
                        o_hi = min(k0 + 256, qs + 512)
                        if o_hi > o_lo:
                            # exp(s+b) == exp(s)*exp(b): banded rel-pos bias
                            # (and causal-mask zeros) as a Pool multiply; the
                            # head's last pair goes on DVE (lower latency)
                            # since the drain waits on it soonest
                            psl = slice(o_lo - qs, o_hi - qs)
                            bsl = slice(o_lo - (k0 - BAND_OFF),
                                        o_hi - (k0 - BAND_OFF))
                            eng = (nc.vector if kt0 + 2 >= nkt(s)
                                   else nc.gpsimd)
                            eng.tensor_tensor(
                                ea[:, j, psl], ea[:, j, psl],
                                band3[:, h, bsl], Alu.mult)
                        ea_map[(s, h, kt)] = (ea[:, j], off)
                        ktc[0] += 1
                        if ktc[0] % 5 == 0:
                            emit_mlp_unit()

                def consume_head(s, h):
                    # drain the whole head qc-major: each query-chunk's psum
                    # accumulation group is contiguous (HW accumulate state
                    # is per-bank, so groups must not interleave)
                    cps = pctx.tile([128, 4, 65], F32, tag="ctx",
                                    name=f"cps{s}_{h}")
                    kmax = nkt(s) - 1
                    for qc in range(4):
                        lastk = min(kmax, 4 * s + qc) if causal else kmax
                        for kt in range(lastk + 1):
                            ea, off = ea_map[(s, h, kt)]
                            nc.tensor.matmul(
                                cps[:, qc, :],
                                ea[:, 128 * qc:128 * qc + 128],
                                va_all[:, 520 * kt + 65 * h:
                                       520 * kt + 65 * h + 65],
                                start=(kt == 0), stop=(kt == lastk))
                    for kt in range(nkt(s)):
                        ea_map.pop((s, h, kt))
                    normalize(s, cps, h)
                    if s > 0:
                        outproj(s - 1, cts_map[s - 1], [divmod(h, 2)])
                        if h == HC - 1:
                            del cts_map[s - 1]

                heads = [(s, h) for s in range(NSUP) for h in range(HC)]
                for j, (s, h) in enumerate(heads):
                    if h == 0:
                        cts_map[s] = [pct.tile([128, 512], BF16, tag="ct",
                                               name=f"ct{s}_{p}")
                                      for p in range(4)]
                    for kt0 in range(0, nkt(s), 2):
                        produce_pair(s, h, kt0)
                        if kt0 == 2:
                            finish_pair()
                            if j >= 1:
                                consume_head(*heads[j - 1])
                consume_head(*heads[-1])
                while pending:
                    finish_pair(force=True)
                outproj(NSUP - 1, cts_map[NSUP - 1],
                        [(qt, ec) for qt in range(4) for ec in range(2)],
                        last=True)
                while mlp_i < len(mlp_units):
                    emit_mlp_unit()

            # ---------------- MLP down-projection ---------------------------
            with (
                tc.tile_pool(name="pobb", bufs=2) as pobb,
                tc.tile_pool(name="pps3", bufs=3, space="PSUM") as pps3,
            ):
                h4 = h8[:].rearrange("p (j pb) l -> p j pb l", j=2)
                wm4 = [w[:].rearrange("p (j pb) e -> p j pb e", j=2)
                       for w in wms]
                for lt in range(LT):
                    ob = pobb.tile([128, 1024], BF16, tag="ob")
                    for ec in range(2):
                        acc = pps3.tile([128, 512], F32, tag="ps3")
                        for ps in range(2):
                            for pb in range(PB):
                                nc.tensor.matmul(
                                    acc[:],
                                    h4[:, :, pb, 128 * lt:128 * lt + 128],
                                    wm4[ps][:, :, pb,
                                            512 * ec:512 * ec + 512],
                                    start=(ps == 0 and pb == 0),
                                    stop=(ps == 1 and pb == PB - 1),
                                    perf_mode=DR)
                        nc.scalar.mul(ob[:, 512 * ec:512 * ec + 512],
                                      acc[:], 1.0 / 32.0)
                        nc.sync.dma_start(
                            mlp_d[128 * lt:128 * lt + 128,
                                  512 * ec:512 * ec + 512],
                            ob[:, 512 * ec:512 * ec + 512])

    nc.compile()
    return nc


_NC_CACHE = {}


def _get_nc(causal: bool):
    if causal not in _NC_CACHE:
        _NC_CACHE[causal] = _build(causal)
    return _NC_CACHE[causal]


def _bucket(n):
    n = np.asarray(n)
    nf = np.maximum(n.astype(np.float32), np.float32(1.0))
    v = np.log(nf / np.float32(16.0)).astype(np.float32)
    v = (v / np.float32(np.log(8.0))) * np.float32(16.0)
    val_large = np.minimum(16 + v.astype(np.int32), NUM_BUCKETS - 1)
    return np.where(n < 16, n, val_large)


def _make_band(rel_emb, heads, causal):
    """exp() of the banded rel-pos bias (causal-masked entries -> 0)."""
    d = np.arange(-(BAND_OFF + 127), 256)
    pos = np.maximum(d, 0)
    bv = rel_emb[_bucket(pos)][:, heads] - rel_emb[NUM_BUCKETS - 1][heads]
    bv = np.where(d[:, None] >= 113, np.float32(0.0), bv)
    bv = np.exp(bv).astype(np.float32)
    if causal:
        bv = np.where(d[:, None] < 0, np.float32(0.0), bv)
    else:
        fut = np.exp(rel_emb[0][heads] - rel_emb[NUM_BUCKETS - 1][heads])
        bv = np.where(d[:, None] < 0, fut[None, :], bv)
    i = np.arange(128)[:, None]
    j = np.arange(BAND_W)[None, :]
    idx = (j - BAND_OFF - i) + (BAND_OFF + 127)
    return bv.astype(np.float32)[idx]          # [128, BAND_W, HC]


def _f8(a):
    return np.ascontiguousarray(a, dtype=np.float32).astype(E4)


def _split16(w, s):
    """-> (e4m3(s*w), e4m3(s*w - f32(e4m3(s*w))), e4m3(f32(e4m3(s*w))/s))"""
    w = np.asarray(w, np.float32)
    s1 = _f8(s * w)
    f1 = s1.astype(np.float32)
    s2 = _f8(s * w - f1)
    s3 = _f8(f1 / s)
    return s1, s2, s3


def _stat_qk(w_c):
    """w_c [E, HC, D] -> [4(tile), 128, TB, 2, 128] in f32 (pre-quant)."""
    arr = w_c.reshape(E, 2, 4, 2, 32)           # e, g, u, dp, dm
    out = np.empty((4, TB, 128, 2, 128), np.float32)
    for tl in range(4):
        g, dp = divmod(tl, 2)
        M = arr[:, g, :, dp, :].reshape(E, 128)  # m = 32u + dm
        out[tl] = M.reshape(TB, 2, 128, 128).transpose(0, 2, 1, 3)
    return out.transpose(0, 2, 1, 3, 4)          # [4, 128, TB, 2, 128]


def _prep_in_maps(inputs, wq, wk, wv, wo, wi, wmo, rel_emb, decoder_mask):
    inputs = np.asarray(inputs, dtype=np.float32)
    wq = np.asarray(wq, dtype=np.float32)
    wk = np.asarray(wk, dtype=np.float32)
    wv = np.asarray(wv, dtype=np.float32)
    wo = np.asarray(wo, dtype=np.float32)
    wi = np.asarray(wi, dtype=np.float32)
    wmo = np.asarray(wmo, dtype=np.float32)
    rel_emb = np.asarray(rel_emb, dtype=np.float32)
    mask = np.asarray(decoder_mask).reshape(L, L)

    tril = np.tril(np.ones((L, L), dtype=bool))
    if np.array_equal(mask, tril):
        causal = True
    elif mask.all():
        causal = False
    else:
        raise NotImplementedError("only causal or all-true masks supported")

    in_maps = []
    for c in range(NCORES):
        b, g = divmod(c, 2)
        heads = np.arange(HC * g, HC * (g + 1))
        band = _make_band(rel_emb, heads, causal)        # [128, W, HC]
        band = np.ascontiguousarray(band.transpose(0, 2, 1)).astype(BF)
        bfut = np.broadcast_to(
            (rel_emb[0][heads] - rel_emb[NUM_BUCKETS - 1][heads])
            .astype(np.float32), (128, HC)).copy()

        xT = inputs[b].T                                  # [E, L]
        x8 = _f8(xT)
        xr = _f8(16.0 * (xT - x8.astype(np.float32)))
        x8 = x8.reshape(ET, 128, L).transpose(1, 0, 2)    # [128, ET, L]
        xr = xr.reshape(ET, 128, L).transpose(1, 0, 2)

        wq_c = wq[:, heads, :]
        wk_c = wk[:, heads, :]
        # 2-pass q/k: keep only (s1, s3) -> [4, 128, 2, TB, 2, 128]
        q1, _, q3 = _split16(_stat_qk(wq_c), 16.0)
        k1, _, k3 = _split16(_stat_qk(wk_c), 16.0)
        wqs = np.ascontiguousarray(
            np.stack([q1, q3], axis=2))                  # [4,128,2,TB,2,128]
        wks = np.ascontiguousarray(np.stack([k1, k3], axis=2))

        wv_c = wv[:, heads, :].reshape(E, HC * D)
        wvm = np.empty((2, 128, 3, TB, 2, 256), E4)
        for vh in range(2):
            N = wv_c[:, 256 * vh:256 * vh + 256]
            N = N.reshape(TB, 2, 128, 256).transpose(2, 0, 1, 3)
            s1, s2, s3 = _split16(N, 16.0)
            wvm[vh, :, 0], wvm[vh, :, 1], wvm[vh, :, 2] = s1, s2, s3

        wi_c = wi[:, FC * g:FC * (g + 1)]
        wis = np.empty((FT // 2, 128, 2, 3, TB, 2, 128), E4)
        for ft in range(FT):
            M = wi_c[:, 128 * ft:128 * ft + 128]
            M = M.reshape(TB, 2, 128, 128).transpose(2, 0, 1, 3)
            s1, s2, s3 = _split16(M, 16.0)
            fp, j = divmod(ft, 2)
            wis[fp, :, j, 0], wis[fp, :, j, 1], wis[fp, :, j, 2] = s1, s2, s3

        wmo_c = wmo[FC * g:FC * (g + 1), :]               # [FC, E]
        wm = wmo_c.reshape(FT, 128, E).transpose(1, 0, 2)  # [128, FT, E]
        m1 = _f8(32.0 * wm)
        m2 = _f8(32.0 * wm - m1.astype(np.float32))
        wmm = np.stack([m1, m2])

        wo_c = wo[heads]                                   # [HC, D, E]
        wos = wo_c.reshape(4, 2, 64, E).transpose(0, 1, 2, 3) \
            .reshape(4, 128, E).transpose(1, 0, 2)         # [128, 4, E]
        wos = np.ascontiguousarray(wos).astype(BF)

        in_maps.append(dict(
            x8=np.ascontiguousarray(x8), xr=np.ascontiguousarray(xr),
            wqs=wqs, wks=wks, wvm=wvm, wis=wis, wmm=wmm,
            wos=wos, band=band, bfut=bfut,
            ident=np.eye(128, dtype=np.float32).astype(BF),
        ))
    return in_maps, causal, inputs


def run(trace=False, **kw):
    in_maps, causal, inputs = _prep_in_maps(**kw)
    nc = _get_nc(causal)
    res = run_bass_kernel_spmd(nc, in_maps, list(range(NCORES)), trace=trace)
    out = np.empty((B, L, E), dtype=np.float32)
    for b in range(B):
        out[b] = (inputs[b]
                  + res.results[2 * b]["attn_out"].astype(np.float32)
                  + res.results[2 * b]["mlp_out"].astype(np.float32)
                  + res.results[2 * b + 1]["attn_out"].astype(np.float32)
                  + res.results[2 * b + 1]["mlp_out"].astype(np.float32))
    return out, res


def kernel(**inputs):
    out, _ = run(**inputs)
    return out


# revision 64
# speedup vs baseline: 1.0123x; 1.0013x over previous
"""Trainium2 Bass kernel for nn_DecoderLayer_19851338842283.

8 cores: data-parallel over batch (4) x tensor-parallel (2) over heads/mlp_dim.
fp8(e4m3) DoubleRow matmuls for projections / MLP (with host-side residual
weight passes for accuracy), fp8-DR d-split scores, bf16 exp/ctx/out-proj.
Host sums the two tensor-parallel partials and adds the residual.

Scheduling/structure (402.7us -> 307.4us on the TimelineSim cost model):
- consolidated DMAs (few big transfers, weights interleaved with x quarters
  so the PE starts ~5us in instead of ~45us)
- q/k projections at 2 fp8 passes (s1@x8 + s3@xr)
- rel-pos band applied as exp(s)*exp(b) on the Pool engine (off PE/DVE)
- two k-tiles of scores share a 2-bank psum tile so one exp instruction
  covers both (halves the Act per-instruction overhead)
- ctx computed in [q, d+1] layout (65-col moving operand: 2.3x less PE time
  than the [d, q] orientation), denominator via the ones column; per-head
  qc-major drains keep psum accumulation groups contiguous per bank (HW
  accumulation state is bank-level - interleaved groups corrupt)
- normalized ctx transposed back to [d, q] with PE transposes (2 heads per
  transpose), deferred a few units to stay off the DVE critical path
- producer (scores/exp/band) runs a full head ahead of the ctx drain;
  MLP-in units interleave into attention at a fixed cadence
- bf16 outputs, halved output DMA traffic
"""

import ml_dtypes
import numpy as np

import concourse.bacc as bacc
import concourse.mybir as mybir
import concourse.tile as tile
from concourse.bass_utils import run_bass_kernel_spmd

F32 = mybir.dt.float32
BF16 = mybir.dt.bfloat16
FP8 = mybir.dt.float8e4
Act = mybir.ActivationFunctionType
Alu = mybir.AluOpType
DR = mybir.MatmulPerfMode.DoubleRow
E4 = ml_dtypes.float8_e4m3
BF = ml_dtypes.bfloat16

B, L, E, H, D, F = 4, 2048, 1024, 16, 64, 4096
HC = H // 2          # heads per core = 8
FC = F // 2          # mlp dim per core = 2048
NCORES = 8
ET = E // 128        # 8
LT = L // 128        # 16
FT = FC // 128       # 16
NSUP = L // 512      # 4
TB = E // 256        # 4 DR pair-blocks over E
PB = FC // 256       # 8 DR pair-blocks over FC
BAND_OFF = 128
BAND_W = 384
NUM_BUCKETS = 32
QSC = np.float32(0.125 ** 0.5 / 16.0)   # psum->q8/k8 copy scale


def _build(causal: bool):
    nc = bacc.Bacc("TRN2", target_bir_lowering=False, debug=False,
                   num_devices=NCORES)
    x8_d = nc.dram_tensor("x8", [128, ET, L], FP8, kind="ExternalInput").ap()
    xr_d = nc.dram_tensor("xr", [128, ET, L], FP8, kind="ExternalInput").ap()
    # q/k weights: [tl, part, ps(2: s1|s3), tb, 2, 128]
    wqs_d = nc.dram_tensor("wqs", [4, 128, 2, TB, 2, 128], FP8,
                           kind="ExternalInput").ap()
    wks_d = nc.dram_tensor("wks", [4, 128, 2, TB, 2, 128], FP8,
                           kind="ExternalInput").ap()
    # v weights: [vh, part, ps(3), tb, 2, 256]
    wvm_d = nc.dram_tensor("wvm", [2, 128, 3, TB, 2, 256], FP8,
                           kind="ExternalInput").ap()
    # wi weights: [ftpair, part, j(2), ps(3), tb, 2, 128]
    wis_d = nc.dram_tensor("wis", [FT // 2, 128, 2, 3, TB, 2, 128], FP8,
                           kind="ExternalInput").ap()
    wmm_d = nc.dram_tensor("wmm", [2, 128, FT, E], FP8,
                           kind="ExternalInput").ap()
    wos_d = nc.dram_tensor("wos", [128, 4, E], BF16, kind="ExternalInput").ap()
    band_d = nc.dram_tensor("band", [128, HC, BAND_W], BF16,
                            kind="ExternalInput").ap()
    ident_d = nc.dram_tensor("ident", [128, 128], BF16,
                             kind="ExternalInput").ap()
    bfut_d = nc.dram_tensor("bfut", [128, HC], F32, kind="ExternalInput").ap()
    attn_d = nc.dram_tensor("attn_out", [L, E], BF16,
                            kind="ExternalOutput").ap()
    mlp_d = nc.dram_tensor("mlp_out", [L, E], BF16,
                           kind="ExternalOutput").ap()

    with tile.TileContext(nc) as tc:
        with (
            tc.tile_pool(name="pbig", bufs=1) as pbig,
            tc.tile_pool(name="pqk", bufs=4) as pqk,
            tc.tile_pool(name="pva", bufs=1) as pva,
        ):
            x8 = pbig.tile([128, ET, L], FP8, tag="x8", name="x8")
            wms = [pbig.tile([128, FT, E], FP8, tag=f"wm{ps}", name=f"wm{ps}")
                   for ps in range(2)]
            xr = pbig.tile([128, ET, L], FP8, tag="xr", name="xr")
            h8 = pbig.tile([128, FT, L], FP8, tag="h8", name="h8")
            q8s = [pqk.tile([128, 2, L], FP8, tag="qk", name=f"q8_{g}")
                   for g in range(2)]
            k8s = [pqk.tile([128, 2, L], FP8, tag="qk", name=f"k8_{g}")
                   for g in range(2)]
            va_all = pva.tile([128, LT * HC * 65], BF16, tag="va", name="va")

            # ---------------- q/k/v projections -----------------------------
            with (
                tc.tile_pool(name="pw", bufs=10) as pw,
                tc.tile_pool(name="pps1", bufs=3, space="PSUM") as pps1,
            ):
                # Allocate all stationary-weight tiles, then issue DMAs in an
                # order that lets the PE start as early as possible: first
                # q-weight tile, then x quarters interleaved with the
                # remaining weight tiles.
                sts_q = [pw.tile([128, 2, TB, 2, 128], FP8, tag="w",
                                 name=f"stq{tl}") for tl in range(4)]
                sts_k = [pw.tile([128, 2, TB, 2, 128], FP8, tag="w",
                                 name=f"stk{tl}") for tl in range(4)]
                wvs = [pw.tile([128, 3, TB, 2, 256], FP8, tag="w",
                               name=f"wv{vh}") for vh in range(2)]

                def xq(c):
                    nc.sync.dma_start(x8[:, :, 512 * c:512 * c + 512],
                                      x8_d[:, :, 512 * c:512 * c + 512])
                    nc.sync.dma_start(xr[:, :, 512 * c:512 * c + 512],
                                      xr_d[:, :, 512 * c:512 * c + 512])

                nc.sync.dma_start(sts_q[0][:], wqs_d[0])
                xq(0)
                nc.sync.dma_start(sts_k[0][:], wks_d[0])
                for tl in range(1, 4):
                    nc.sync.dma_start(sts_q[tl][:], wqs_d[tl])
                    nc.sync.dma_start(sts_k[tl][:], wks_d[tl])
                for c in range(1, NSUP):
                    xq(c)
                for vh in range(2):
                    nc.sync.dma_start(wvs[vh][:], wvm_d[vh])
                for ps in range(2):
                    nc.sync.dma_start(wms[ps][:], wmm_d[ps])

                # q/k: 2 passes (s1 @ x8 + s3 @ xr)
                def xmov2(ps, tb, c0, cw):
                    src = x8 if ps == 0 else xr
                    return src[:, 2 * tb:2 * tb + 2, c0:c0 + cw]

                # v / wi: 3 passes (s1 @ x8 + s2 @ x8 + s3 @ xr)
                def xmov3(ps, tb, c0, cw):
                    src = x8 if ps < 2 else xr
                    return src[:, 2 * tb:2 * tb + 2, c0:c0 + cw]

                # c-major so the PE consumes x quarters in DMA-arrival order;
                # q/k alternated to match the weight-DMA issue order
                for c in range(NSUP):
                    for tl in range(4):          # (g, dpair)
                        for sts, dsts in ((sts_q, q8s), (sts_k, k8s)):
                            g, dp = divmod(tl, 2)
                            st = sts[tl]
                            acc = pps1.tile([128, 512], F32, tag="ps1")
                            for ps in range(2):
                                for tb in range(TB):
                                    nc.tensor.matmul(
                                        acc[:], st[:, ps, tb],
                                        xmov2(ps, tb, 512 * c, 512),
                                        start=(ps == 0 and tb == 0),
                                        stop=(ps == 1 and tb == TB - 1),
                                        perf_mode=DR)
                            nc.scalar.mul(
                                dsts[g][:, dp, 512 * c:512 * c + 512],
                                acc[:], float(QSC))

                # ---------------- v projection (bf16 va + ones) -------------
                ones_c = nc.const_aps.tensor(1.0, [128, HC, 1], BF16)
                for vh in range(2):
                    wv = wvs[vh]
                    for lt in range(LT):
                        acc = pps1.tile([128, 256], F32, tag="ps1")
                        for ps in range(3):
                            for tb in range(TB):
                                nc.tensor.matmul(
                                    acc[:], xmov3(ps, tb, 128 * lt, 128),
                                    wv[:, ps, tb],
                                    start=(ps == 0 and tb == 0),
                                    stop=(ps == 2 and tb == TB - 1),
                                    perf_mode=DR)
                        va3 = va_all[:, 520 * lt:520 * lt + 520] \
                            .rearrange("p (h c) -> p h c", h=HC)
                        nc.vector.tensor_scalar_mul(
                            va3[:, 4 * vh:4 * vh + 4, 0:64],
                            acc[:].rearrange("p (h c) -> p h c", h=4),
                            1.0 / 16.0)
                        if vh == 0:
                            nc.vector.tensor_copy(va3[:, :, 64:65], ones_c)

            # -------- attention (+ interleaved MLP-in) ----------------------
            with (
                tc.tile_pool(name="pwi", bufs=3) as pwi,
                tc.tile_pool(name="pband", bufs=1) as pband,
                tc.tile_pool(name="pct", bufs=12) as pct,
                tc.tile_pool(name="pwo", bufs=1) as pwo,
                tc.tile_pool(name="pexp", bufs=13) as pexp,
                tc.tile_pool(name="prr", bufs=2) as prr,
                tc.tile_pool(name="pcsb", bufs=3) as pcsb,
                tc.tile_pool(name="poba", bufs=2) as poba,
                tc.tile_pool(name="pps", bufs=2, space="PSUM") as pps,
                tc.tile_pool(name="pctx", bufs=1, space="PSUM") as pctx,
                tc.tile_pool(name="pasm", bufs=1, space="PSUM") as pasm,
                tc.tile_pool(name="pout", bufs=1, space="PSUM") as pout,
                tc.tile_pool(name="pmps", bufs=1, space="PSUM") as pmps,
            ):
                band_sb = pband.tile([128, HC * BAND_W], BF16, tag="band")
                band3 = band_sb[:].rearrange("p (h w) -> p h w", h=HC)
                nc.sync.dma_start(band3, band_d[:, :, :])
                ident = pband.tile([128, 128], BF16, tag="ident")
                nc.sync.dma_start(ident[:], ident_d)
                bfut_sb = pband.tile([128, HC], F32, tag="bfut")
                if not causal:
                    nc.sync.dma_start(bfut_sb[:], bfut_d)
                wos = pwo.tile([128, 4, E], BF16, tag="wo", name="wos")
                nc.sync.dma_start(wos[:], wos_d)

                mlp_units = [(ft, c) for ft in range(FT) for c in range(NSUP)]
                mo_units = [(lt, ec) for lt in range(LT) for ec in range(2)]
                h4a = h8[:].rearrange("p (j pb) l -> p j pb l", j=2)
                wm4a = [w[:].rearrange("p (j pb) e -> p j pb e", j=2)
                        for w in wms]

                mlp_i = 0
                wi_sts = {}

                def wi_fetch(fp):
                    if fp >= FT // 2 or fp in wi_sts:
                        return
                    t = pwi.tile([128, 2, 3, TB, 2, 128], FP8, tag="wi",
                                 name=f"wist{fp}")
                    nc.sync.dma_start(t[:], wis_d[fp])
                    wi_sts[fp] = t

                wi_fetch(0)
                wi_fetch(1)
                wi_fetch(2)

                def emit_mlp_unit():
                    nonlocal mlp_i
                    if mlp_i >= len(mlp_units):
                        return
                    ft, c = mlp_units[mlp_i]
                    mlp_i += 1
                    if c == 0:
                        wi_fetch(ft // 2 + 2)
                    t = wi_sts[ft // 2]
                    acc = pmps.tile([128, 512], F32, tag="mps")
                    for ps in range(3):
                        for tb in range(TB):
                            nc.tensor.matmul(
                                acc[:], t[:, ft % 2, ps, tb],
                                xmov3(ps, tb, 512 * c, 512),
                                start=(ps == 0 and tb == 0),
                                stop=(ps == 2 and tb == TB - 1),
                                perf_mode=DR)
                    nc.vector.tensor_scalar(
                        out=h8[:, ft, 512 * c:512 * c + 512], in0=acc[:],
                        scalar1=1.0 / 16.0, scalar2=0.0,
                        op0=Alu.mult, op1=Alu.max)

                ob_tiles = {}

                def outproj(s_prev, cts_prev, chunks, last=False):
                    for qt, ec in chunks:
                        acc = pout.tile([128, 512], F32, tag="out",
                                        name="opacc")
                        for p in range(4):
                            nc.tensor.matmul(
                                acc[:],
                                cts_prev[p][:, 128 * qt:128 * qt + 128],
                                wos[:, p, 512 * ec:512 * ec + 512],
                                start=(p == 0), stop=(p == 3))
                        if ec == 0:
                            ob_tiles[qt] = poba.tile([128, 1024], BF16,
                                                     tag="ob",
                                                     name=f"ob{s_prev}_{qt}")
                        ob = ob_tiles[qt]
                        if last and ec == 1 and qt == 3:
                            nc.scalar.copy(ob[:, 512 * ec:512 * ec + 512],
                                           acc[:])
                        else:
                            nc.vector.tensor_copy(
                                ob[:, 512 * ec:512 * ec + 512], acc[:])
                        if ec == 1:
                            qs0 = 512 * s_prev
                            nc.sync.dma_start(
                                attn_d[qs0 + 128 * qt:qs0 + 128 * qt + 128,
                                       :], ob[:])

                def nkt(s):
                    return 4 * (s + 1) if causal else LT

                cts_map = {}
                csb_map = {}
                ea_map = {}
                ktc = [0]

                pending = []     # deferred transpose+copy blocks

                def normalize(s, cps, h):
                    # cps [128, 4, 65]: per-chunk scale by 1/denominator
                    p = h // 2
                    if h % 2 == 0:
                        csb_map[(s, p)] = pcsb.tile([128, 4, 2, 64], BF16,
                                                    tag="csb",
                                                    name=f"csb{s}_{p}")
                    csb = csb_map[(s, p)]
                    rr = prr.tile([128, 4, 1], F32, tag="rr", name="rr")
                    nc.vector.reciprocal(rr[:], cps[:, :, 64:65])
                    for qc in range(4):
                        nc.vector.tensor_scalar_mul(
                            csb[:, qc, h % 2, :], cps[:, qc, 0:64],
                            rr[:, qc])
                    if h % 2 == 1:
                        # both heads of p normalized: defer the PE transposes
                        # so they don't stall on the DVE normalize above
                        pending.append((s, p, csb, cts_map[s]))
                        del csb_map[(s, p)]

                def finish_pair(force=False):
                    # depth-1 deferral only: outproj chunks read all four
                    # cts[p] tiles, so the last pair's transposes must land
                    # before the next supertile's first outproj chunk
                    if not pending:
                        return
                    s, p, csb, cts = pending.pop(0)
                    asm = pasm.tile([128, 512], BF16, tag="asm",
                                    name=f"asm{s}_{p}")
                    for qc in range(4):
                        nc.tensor.transpose(
                            asm[:, 128 * qc:128 * qc + 128],
                            csb[:, qc, :, :].rearrange("p a b -> p (a b)"),
                            ident[:])
                    nc.vector.tensor_copy(cts[p][:], asm[:])

                def produce_pair(s, h, kt0):
                    # two k-tiles (kt0, kt0+1) share one 2-bank psum tile so
                    # a single exp instruction covers both (halving the Act
                    # per-instruction overhead); the diagonal pairs keep two
                    # exps to skip the masked region
                    qs = 512 * s
                    g, u = divmod(h, 4)
                    psA = pps.tile([128, 2, 512], F32, tag="ps", name="sc")
                    ea = pexp.tile([128, 2, 512], BF16, tag="exp", name="ea")
                    offs = []
                    for j in range(2):
                        kt = kt0 + j
                        k0 = 128 * kt
                        off = min(max(0, k0 - qs), 384) if causal else 0
                        offs.append(off)
                        nc.tensor.matmul(
                            psA[:, j, off:512],
                            k8s[g][32 * u:32 * u + 32, :, k0:k0 + 128],
                            q8s[g][32 * u:32 * u + 32, :, qs + off:qs + 512],
                            start=True, stop=True,
                            perf_mode=DR, tile_position=(32 * u, 0))
                        ul = (min(max(k0 - BAND_OFF - qs, 0), 512)
                              if not causal else 0)
                        if ul > 0:
                            nc.vector.tensor_scalar_add(
                                psA[:, j, 0:ul], psA[:, j, 0:ul],
                                bfut_sb[:, h:h + 1])
                    psF = psA[:].rearrange("p a b -> p (a b)")
                    eaF = ea[:].rearrange("p a b -> p (a b)")
                    if offs[1] == 0:
                        nc.scalar.activation(eaF[:, 0:1024], psF[:, 0:1024],
                                             Act.Exp)
                    else:
                        nc.scalar.activation(eaF[:, offs[0]:512],
                                             psF[:, offs[0]:512], Act.Exp)
                        nc.scalar.activation(eaF[:, 512 + offs[1]:1024],
                                             psF[:, 512 + offs[1]:1024],
                                             Act.Exp)
                    for j in range(2):
                        kt = kt0 + j
                        k0 = 128 * kt
                        off = offs[j]
                        o_lo = max(k0 - BAND_OFF, qs + off)
                        o_hi = min(k0 + 256, qs + 512)
                        if o_hi > o_lo:
                            # exp(s+b) == exp(s)*exp(b): banded rel-pos bias
                            # (and causal-mask zeros) as a Pool multiply; the
                            # head's last pair goes on DVE (lower latency)
                            # since the drain waits on it soonest
                            psl = slice(o_lo - qs, o_hi - qs)
                            bsl = slice(o_lo - (k0 - BAND_OFF),
                                        o_hi - (k0 - BAND_OFF))
                            eng = (nc.vector if kt0 + 2 >= nkt(s)
                                   else nc.gpsimd)
                            eng.tensor_tensor(
                                ea[:, j, psl], ea[:, j, psl],
                                band3[:, h, bsl], Alu.mult)
                        ea_map[(s, h, kt)] = (ea[:, j], off)
                        ktc[0] += 1
                        if ktc[0] % 5 == 0:
                            emit_mlp_unit()

                def consume_head(s, h):
                    # drain the whole head qc-major: each query-chunk's psum
                    # accumulation group is contiguous (HW accumulate state
                    # is per-bank, so groups must not interleave)
                    cps = pctx.tile([128, 4, 65], F32, tag="ctx",
                                    name=f"cps{s}_{h}")
                    kmax = nkt(s) - 1
                    for qc in range(4):
                        lastk = min(kmax, 4 * s + qc) if causal else kmax
                        for kt in range(lastk + 1):
                            ea, off = ea_map[(s, h, kt)]
                            nc.tensor.matmul(
                                cps[:, qc, :],
                                ea[:, 128 * qc:128 * qc + 128],
                                va_all[:, 520 * kt + 65 * h:
                                       520 * kt + 65 * h + 65],
                                start=(kt == 0), stop=(kt == lastk))
                    for kt in range(nkt(s)):
                        ea_map.pop((s, h, kt))
                    normalize(s, cps, h)
                    if s > 0:
                        outproj(s - 1, cts_map[s - 1], [divmod(h, 2)])
                        if h == HC - 1:
                            del cts_map[s - 1]

                heads = [(s, h) for s in range(NSUP) for h in range(HC)]
                for j, (s, h) in enumerate(heads):
                    if h == 0:
                        cts_map[s] = [pct.tile([128, 512], BF16, tag="ct",
                                               name=f"ct{s}_{p}")
                                      for p in range(4)]
                    for kt0 in range(0, nkt(s), 2):
                        produce_pair(s, h, kt0)
                        if kt0 == 2:
                            finish_pair()
                            if j >= 1:
                                consume_head(*heads[j - 1])
                consume_head(*heads[-1])
                while pending:
                    finish_pair(force=True)
                outproj(NSUP - 1, cts_map[NSUP - 1],
                        [(qt, ec) for qt in range(4) for ec in range(2)],
                        last=True)
                while mlp_i < len(mlp_units):
                    emit_mlp_unit()

            # ---------------- MLP down-projection ---------------------------
            with (
                tc.tile_pool(name="pobb", bufs=2) as pobb,
                tc.tile_pool(name="pps3", bufs=3, space="PSUM") as pps3,
            ):
                h4 = h8[:].rearrange("p (j pb) l -> p j pb l", j=2)
                wm4 = [w[:].rearrange("p (j pb) e -> p j pb e", j=2)
                       for w in wms]
                for lt in range(LT):
                    ob = pobb.tile([128, 1024], BF16, tag="ob")
                    for ec in range(2):
                        # the very last unit splits into two independent
                        # 256-col accs so its first half's copy+DMA overlaps
                        # the second half's matmuls, shortening the final
                        # drain chain
                        nch = 2 if (lt == LT - 1 and ec == 1) else 1
                        w = 512 // nch
                        for ch in range(nch):
                            c0 = 512 * ec + ch * w
                            acc = pps3.tile([128, w], F32, tag="ps3",
                                            name="acc3")
                            for ps in range(2):
                                for pb in range(PB):
                                    nc.tensor.matmul(
                                        acc[:],
                                        h4[:, :, pb,
                                           128 * lt:128 * lt + 128],
                                        wm4[ps][:, :, pb, c0:c0 + w],
                                        start=(ps == 0 and pb == 0),
                                        stop=(ps == 1 and pb == PB - 1),
                                        perf_mode=DR)
                            nc.scalar.mul(ob[:, c0:c0 + w], acc[:],
                                          1.0 / 32.0)
                            nc.sync.dma_start(
                                mlp_d[128 * lt:128 * lt + 128, c0:c0 + w],
                                ob[:, c0:c0 + w])

    nc.compile()
    return nc


_NC_CACHE = {}


def _get_nc(causal: bool):
    if causal not in _NC_CACHE:
        _NC_CACHE[causal] = _build(causal)
    return _NC_CACHE[causal]


def _bucket(n):
    n = np.asarray(n)
    nf = np.maximum(n.astype(np.float32), np.float32(1.0))
    v = np.log(nf / np.float32(16.0)).astype(np.float32)
    v = (v / np.float32(np.log(8.0))) * np.float32(16.0)
    val_large = np.minimum(16 + v.astype(np.int32), NUM_BUCKETS - 1)
    return np.where(n < 16, n, val_large)


def _make_band(rel_emb, heads, causal):
    """exp() of the banded rel-pos bias (causal-masked entries -> 0)."""
    d = np.arange(-(BAND_OFF + 127), 256)
    pos = np.maximum(d, 0)
    bv = rel_emb[_bucket(pos)][:, heads] - rel_emb[NUM_BUCKETS - 1][heads]
    bv = np.where(d[:, None] >= 113, np.float32(0.0), bv)
    bv = np.exp(bv).astype(np.float32)
    if causal:
        bv = np.where(d[:, None] < 0, np.float32(0.0), bv)
    else:
        fut = np.exp(rel_emb[0][heads] - rel_emb[NUM_BUCKETS - 1][heads])
        bv = np.where(d[:, None] < 0, fut[None, :], bv)
    i = np.arange(128)[:, None]
    j = np.arange(BAND_W)[None, :]
    idx = (j - BAND_OFF - i) + (BAND_OFF + 127)
    return bv.astype(np.float32)[idx]          # [128, BAND_W, HC]


def _f8(a):
    return np.ascontiguousarray(a, dtype=np.float32).astype(E4)


def _split16(w, s):
    """-> (e4m3(s*w), e4m3(s*w - f32(e4m3(s*w))), e4m3(f32(e4m3(s*w))/s))"""
    w = np.asarray(w, np.float32)
    s1 = _f8(s * w)
    f1 = s1.astype(np.float32)
    s2 = _f8(s * w - f1)
    s3 = _f8(f1 / s)
    return s1, s2, s3


def _stat_qk(w_c):
    """w_c [E, HC, D] -> [4(tile), 128, TB, 2, 128] in f32 (pre-quant)."""
    arr = w_c.reshape(E, 2, 4, 2, 32)           # e, g, u, dp, dm
    out = np.empty((4, TB, 128, 2, 128), np.float32)
    for tl in range(4):
        g, dp = divmod(tl, 2)
        M = arr[:, g, :, dp, :].reshape(E, 128)  # m = 32u + dm
        out[tl] = M.reshape(TB, 2, 128, 128).transpose(0, 2, 1, 3)
    return out.transpose(0, 2, 1, 3, 4)          # [4, 128, TB, 2, 128]


def _prep_in_maps(inputs, wq, wk, wv, wo, wi, wmo, rel_emb, decoder_mask):
    inputs = np.asarray(inputs, dtype=np.float32)
    wq = np.asarray(wq, dtype=np.float32)
    wk = np.asarray(wk, dtype=np.float32)
    wv = np.asarray(wv, dtype=np.float32)
    wo = np.asarray(wo, dtype=np.float32)
    wi = np.asarray(wi, dtype=np.float32)
    wmo = np.asarray(wmo, dtype=np.float32)
    rel_emb = np.asarray(rel_emb, dtype=np.float32)
    mask = np.asarray(decoder_mask).reshape(L, L)

    tril = np.tril(np.ones((L, L), dtype=bool))
    if np.array_equal(mask, tril):
        causal = True
    elif mask.all():
        causal = False
    else:
        raise NotImplementedError("only causal or all-true masks supported")

    in_maps = []
    for c in range(NCORES):
        b, g = divmod(c, 2)
        heads = np.arange(HC * g, HC * (g + 1))
        band = _make_band(rel_emb, heads, causal)        # [128, W, HC]
        band = np.ascontiguousarray(band.transpose(0, 2, 1)).astype(BF)
        bfut = np.broadcast_to(
            (rel_emb[0][heads] - rel_emb[NUM_BUCKETS - 1][heads])
            .astype(np.float32), (128, HC)).copy()

        xT = inputs[b].T                                  # [E, L]
        x8 = _f8(xT)
        xr = _f8(16.0 * (xT - x8.astype(np.float32)))
        x8 = x8.reshape(ET, 128, L).transpose(1, 0, 2)    # [128, ET, L]
        xr = xr.reshape(ET, 128, L).transpose(1, 0, 2)

        wq_c = wq[:, heads, :]
        wk_c = wk[:, heads, :]
        # 2-pass q/k: keep only (s1, s3) -> [4, 128, 2, TB, 2, 128]
        q1, _, q3 = _split16(_stat_qk(wq_c), 16.0)
        k1, _, k3 = _split16(_stat_qk(wk_c), 16.0)
        wqs = np.ascontiguousarray(
            np.stack([q1, q3], axis=2))                  # [4,128,2,TB,2,128]
        wks = np.ascontiguousarray(np.stack([k1, k3], axis=2))

        wv_c = wv[:, heads, :].reshape(E, HC * D)
        wvm = np.empty((2, 128, 3, TB, 2, 256), E4)
        for vh in range(2):
            N = wv_c[:, 256 * vh:256 * vh + 256]
            N = N.reshape(TB, 2, 128, 256).transpose(2, 0, 1, 3)
            s1, s2, s3 = _split16(N, 16.0)
            wvm[vh, :, 0], wvm[vh, :, 1], wvm[vh, :, 2] = s1, s2, s3

        wi_c = wi[:, FC * g:FC * (g + 1)]
        wis = np.empty((FT // 2, 128, 2, 3, TB, 2, 128), E4)
        for ft in range(FT):
            M = wi_c[:, 128 * ft:128 * ft + 128]
            M = M.reshape(TB, 2, 128, 128).transpose(2, 0, 1, 3)
            s1, s2, s3 = _split16(M, 16.0)
            fp, j = divmod(ft, 2)
            wis[fp, :, j, 0], wis[fp, :, j, 1], wis[fp, :, j, 2] = s1, s2, s3

        wmo_c = wmo[FC * g:FC * (g + 1), :]               # [FC, E]
        wm = wmo_c.reshape(FT, 128, E).transpose(1, 0, 2)  # [128, FT, E]
        m1 = _f8(32.0 * wm)
        m2 = _f8(32.0 * wm - m1.astype(np.float32))
        wmm = np.stack([m1, m2])

        wo_c = wo[heads]                                   # [HC, D, E]
        wos = wo_c.reshape(4, 2, 64, E).transpose(0, 1, 2, 3) \
            .reshape(4, 128, E).transpose(1, 0, 2)         # [128, 4, E]
        wos = np.ascontiguousarray(wos).astype(BF)

        in_maps.append(dict(
            x8=np.ascontiguousarray(x8), xr=np.ascontiguousarray(xr),
            wqs=wqs, wks=wks, wvm=wvm, wis=wis, wmm=wmm,
            wos=wos, band=band, bfut=bfut,
            ident=np.eye(128, dtype=np.float32).astype(BF),
        ))
    return in_maps, causal, inputs


def run(trace=False, **kw):
    in_maps, causal, inputs = _prep_in_maps(**kw)
    nc = _get_nc(causal)
    res = run_bass_kernel_spmd(nc, in_maps, list(range(NCORES)), trace=trace)
    out = np.empty((B, L, E), dtype=np.float32)
    for b in range(B):
        out[b] = (inputs[b]
                  + res.results[2 * b]["attn_out"].astype(np.float32)
                  + res.results[2 * b]["mlp_out"].astype(np.float32)
                  + res.results[2 * b + 1]["attn_out"].astype(np.float32)
                  + res.results[2 * b + 1]["mlp_out"].astype(np.float32))
    return out, res


def kernel(**inputs):
    out, _ = run(**inputs)
    return out


# revision 66
# speedup vs baseline: 1.0128x; 1.0004x over previous
"""Trainium2 Bass kernel for nn_DecoderLayer_19851338842283.

8 cores: data-parallel over batch (4) x tensor-parallel (2) over heads/mlp_dim.
fp8(e4m3) DoubleRow matmuls for projections / MLP (with host-side residual
weight passes for accuracy), fp8-DR d-split scores, bf16 exp/ctx/out-proj.
Host sums the two tensor-parallel partials and adds the residual.

Scheduling/structure (402.7us -> 307.4us on the TimelineSim cost model):
- consolidated DMAs (few big transfers, weights interleaved with x quarters
  so the PE starts ~5us in instead of ~45us)
- q/k projections at 2 fp8 passes (s1@x8 + s3@xr)
- rel-pos band applied as exp(s)*exp(b) on the Pool engine (off PE/DVE)
- two k-tiles of scores share a 2-bank psum tile so one exp instruction
  covers both (halves the Act per-instruction overhead)
- ctx computed in [q, d+1] layout (65-col moving operand: 2.3x less PE time
  than the [d, q] orientation), denominator via the ones column; per-head
  qc-major drains keep psum accumulation groups contiguous per bank (HW
  accumulation state is bank-level - interleaved groups corrupt)
- normalized ctx transposed back to [d, q] with PE transposes (2 heads per
  transpose), deferred a few units to stay off the DVE critical path
- producer (scores/exp/band) runs a full head ahead of the ctx drain;
  MLP-in units interleave into attention at a fixed cadence
- bf16 outputs, halved output DMA traffic
"""

import ml_dtypes
import numpy as np

import concourse.bacc as bacc
import concourse.mybir as mybir
import concourse.tile as tile
from concourse.bass_utils import run_bass_kernel_spmd

F32 = mybir.dt.float32
BF16 = mybir.dt.bfloat16
FP8 = mybir.dt.float8e4
Act = mybir.ActivationFunctionType
Alu = mybir.AluOpType
DR = mybir.MatmulPerfMode.DoubleRow
E4 = ml_dtypes.float8_e4m3
BF = ml_dtypes.bfloat16

B, L, E, H, D, F = 4, 2048, 1024, 16, 64, 4096
HC = H // 2          # heads per core = 8
FC = F // 2          # mlp dim per core = 2048
NCORES = 8
ET = E // 128        # 8
LT = L // 128        # 16
FT = FC // 128       # 16
NSUP = L // 512      # 4
TB = E // 256        # 4 DR pair-blocks over E
PB = FC // 256       # 8 DR pair-blocks over FC
BAND_OFF = 128
BAND_W = 384
NUM_BUCKETS = 32
QSC = np.float32(0.125 ** 0.5 / 16.0)   # psum->q8/k8 copy scale


def _build(causal: bool):
    nc = bacc.Bacc("TRN2", target_bir_lowering=False, debug=False,
                   num_devices=NCORES)
    x8_d = nc.dram_tensor("x8", [128, ET, L], FP8, kind="ExternalInput").ap()
    xr_d = nc.dram_tensor("xr", [128, ET, L], FP8, kind="ExternalInput").ap()
    # q/k weights: [tl, part, ps(2: s1|s3), tb, 2, 128]
    wqs_d = nc.dram_tensor("wqs", [4, 128, 2, TB, 2, 128], FP8,
                           kind="ExternalInput").ap()
    wks_d = nc.dram_tensor("wks", [4, 128, 2, TB, 2, 128], FP8,
                           kind="ExternalInput").ap()
    # v weights: [vh, part, ps(3), tb, 2, 256]
    wvm_d = nc.dram_tensor("wvm", [2, 128, 3, TB, 2, 256], FP8,
                           kind="ExternalInput").ap()
    # wi weights: [ftpair, part, j(2), ps(3), tb, 2, 128]
    wis_d = nc.dram_tensor("wis", [FT // 2, 128, 2, 3, TB, 2, 128], FP8,
                           kind="ExternalInput").ap()
    wmm_d = nc.dram_tensor("wmm", [2, 128, FT, E], FP8,
                           kind="ExternalInput").ap()
    wos_d = nc.dram_tensor("wos", [128, 4, E], BF16, kind="ExternalInput").ap()
    band_d = nc.dram_tensor("band", [128, HC, BAND_W], BF16,
                            kind="ExternalInput").ap()
    ident_d = nc.dram_tensor("ident", [128, 128], BF16,
                             kind="ExternalInput").ap()
    bfut_d = nc.dram_tensor("bfut", [128, HC], F32, kind="ExternalInput").ap()
    attn_d = nc.dram_tensor("attn_out", [L, E], BF16,
                            kind="ExternalOutput").ap()
    mlp_d = nc.dram_tensor("mlp_out", [L, E], BF16,
                           kind="ExternalOutput").ap()

    with tile.TileContext(nc) as tc:
        with (
            tc.tile_pool(name="pbig", bufs=1) as pbig,
            tc.tile_pool(name="pqk", bufs=4) as pqk,
            tc.tile_pool(name="pva", bufs=1) as pva,
        ):
            x8 = pbig.tile([128, ET, L], FP8, tag="x8", name="x8")
            wms = [pbig.tile([128, FT, E], FP8, tag=f"wm{ps}", name=f"wm{ps}")
                   for ps in range(2)]
            xr = pbig.tile([128, ET, L], FP8, tag="xr", name="xr")
            h8 = pbig.tile([128, FT, L], FP8, tag="h8", name="h8")
            q8s = [pqk.tile([128, 2, L], FP8, tag="qk", name=f"q8_{g}")
                   for g in range(2)]
            k8s = [pqk.tile([128, 2, L], FP8, tag="qk", name=f"k8_{g}")
                   for g in range(2)]
            va_all = pva.tile([128, LT * HC * 65], BF16, tag="va", name="va")

            # ---------------- q/k/v projections -----------------------------
            with (
                tc.tile_pool(name="pw", bufs=10) as pw,
                tc.tile_pool(name="pps1", bufs=3, space="PSUM") as pps1,
            ):
                # Allocate all stationary-weight tiles, then issue DMAs in an
                # order that lets the PE start as early as possible: first
                # q-weight tile, then x quarters interleaved with the
                # remaining weight tiles.
                sts_q = [pw.tile([128, 2, TB, 2, 128], FP8, tag="w",
                                 name=f"stq{tl}") for tl in range(4)]
                sts_k = [pw.tile([128, 2, TB, 2, 128], FP8, tag="w",
                                 name=f"stk{tl}") for tl in range(4)]
                wvs = [pw.tile([128, 3, TB, 2, 256], FP8, tag="w",
                               name=f"wv{vh}") for vh in range(2)]

                def xq(c):
                    nc.sync.dma_start(x8[:, :, 512 * c:512 * c + 512],
                                      x8_d[:, :, 512 * c:512 * c + 512])
                    nc.sync.dma_start(xr[:, :, 512 * c:512 * c + 512],
                                      xr_d[:, :, 512 * c:512 * c + 512])

                nc.sync.dma_start(sts_q[0][:], wqs_d[0])
                xq(0)
                nc.sync.dma_start(sts_k[0][:], wks_d[0])
                for tl in range(1, 4):
                    nc.sync.dma_start(sts_q[tl][:], wqs_d[tl])
                    nc.sync.dma_start(sts_k[tl][:], wks_d[tl])
                for c in range(1, NSUP):
                    xq(c)
                for vh in range(2):
                    nc.sync.dma_start(wvs[vh][:], wvm_d[vh])
                for ps in range(2):
                    nc.sync.dma_start(wms[ps][:], wmm_d[ps])

                # q/k: 2 passes (s1 @ x8 + s3 @ xr)
                def xmov2(ps, tb, c0, cw):
                    src = x8 if ps == 0 else xr
                    return src[:, 2 * tb:2 * tb + 2, c0:c0 + cw]

                # v / wi: 3 passes (s1 @ x8 + s2 @ x8 + s3 @ xr)
                def xmov3(ps, tb, c0, cw):
                    src = x8 if ps < 2 else xr
                    return src[:, 2 * tb:2 * tb + 2, c0:c0 + cw]

                # c-major so the PE consumes x quarters in DMA-arrival order;
                # q/k alternated to match the weight-DMA issue order
                for c in range(NSUP):
                    for tl in range(4):          # (g, dpair)
                        for sts, dsts in ((sts_q, q8s), (sts_k, k8s)):
                            g, dp = divmod(tl, 2)
                            st = sts[tl]
                            acc = pps1.tile([128, 512], F32, tag="ps1")
                            for ps in range(2):
                                for tb in range(TB):
                                    nc.tensor.matmul(
                                        acc[:], st[:, ps, tb],
                                        xmov2(ps, tb, 512 * c, 512),
                                        start=(ps == 0 and tb == 0),
                                        stop=(ps == 1 and tb == TB - 1),
                                        perf_mode=DR)
                            nc.scalar.mul(
                                dsts[g][:, dp, 512 * c:512 * c + 512],
                                acc[:], float(QSC))

                # ---------------- v projection (bf16 va + ones) -------------
                ones_c = nc.const_aps.tensor(1.0, [128, HC, 1], BF16)
                for vh in range(2):
                    wv = wvs[vh]
                    for lt in range(LT):
                        acc = pps1.tile([128, 256], F32, tag="ps1")
                        for ps in range(3):
                            for tb in range(TB):
                                nc.tensor.matmul(
                                    acc[:], xmov3(ps, tb, 128 * lt, 128),
                                    wv[:, ps, tb],
                                    start=(ps == 0 and tb == 0),
                                    stop=(ps == 2 and tb == TB - 1),
                                    perf_mode=DR)
                        va3 = va_all[:, 520 * lt:520 * lt + 520] \
                            .rearrange("p (h c) -> p h c", h=HC)
                        nc.vector.tensor_scalar_mul(
                            va3[:, 4 * vh:4 * vh + 4, 0:64],
                            acc[:].rearrange("p (h c) -> p h c", h=4),
                            1.0 / 16.0)
                        if vh == 0:
                            nc.vector.tensor_copy(va3[:, :, 64:65], ones_c)

            # -------- attention (+ interleaved MLP-in) ----------------------
            with (
                tc.tile_pool(name="pwi", bufs=3) as pwi,
                tc.tile_pool(name="pband", bufs=1) as pband,
                tc.tile_pool(name="pct", bufs=12) as pct,
                tc.tile_pool(name="pwo", bufs=1) as pwo,
                tc.tile_pool(name="pexp", bufs=14) as pexp,
                tc.tile_pool(name="prr", bufs=2) as prr,
                tc.tile_pool(name="pcsb", bufs=3) as pcsb,
                tc.tile_pool(name="poba", bufs=2) as poba,
                tc.tile_pool(name="pps", bufs=2, space="PSUM") as pps,
                tc.tile_pool(name="pctx", bufs=1, space="PSUM") as pctx,
                tc.tile_pool(name="pasm", bufs=1, space="PSUM") as pasm,
                tc.tile_pool(name="pout", bufs=1, space="PSUM") as pout,
                tc.tile_pool(name="pmps", bufs=1, space="PSUM") as pmps,
            ):
                band_sb = pband.tile([128, HC * BAND_W], BF16, tag="band")
                band3 = band_sb[:].rearrange("p (h w) -> p h w", h=HC)
                nc.sync.dma_start(band3, band_d[:, :, :])
                ident = pband.tile([128, 128], BF16, tag="ident")
                nc.sync.dma_start(ident[:], ident_d)
                bfut_sb = pband.tile([128, HC], F32, tag="bfut")
                if not causal:
                    nc.sync.dma_start(bfut_sb[:], bfut_d)
                wos = pwo.tile([128, 4, E], BF16, tag="wo", name="wos")
                nc.sync.dma_start(wos[:], wos_d)

                mlp_units = [(ft, c) for ft in range(FT) for c in range(NSUP)]
                mo_units = [(lt, ec) for lt in range(LT) for ec in range(2)]
                h4a = h8[:].rearrange("p (j pb) l -> p j pb l", j=2)
                wm4a = [w[:].rearrange("p (j pb) e -> p j pb e", j=2)
                        for w in wms]

                mlp_i = 0
                wi_sts = {}

                def wi_fetch(fp):
                    if fp >= FT // 2 or fp in wi_sts:
                        return
                    t = pwi.tile([128, 2, 3, TB, 2, 128], FP8, tag="wi",
                                 name=f"wist{fp}")
                    nc.sync.dma_start(t[:], wis_d[fp])
                    wi_sts[fp] = t

                wi_fetch(0)
                wi_fetch(1)
                wi_fetch(2)

                def emit_mlp_unit():
                    nonlocal mlp_i
                    if mlp_i >= len(mlp_units):
                        return
                    ft, c = mlp_units[mlp_i]
                    mlp_i += 1
                    if c == 0:
                        wi_fetch(ft // 2 + 2)
                    t = wi_sts[ft // 2]
                    acc = pmps.tile([128, 512], F32, tag="mps")
                    for ps in range(3):
                        for tb in range(TB):
                            nc.tensor.matmul(
                                acc[:], t[:, ft % 2, ps, tb],
                                xmov3(ps, tb, 512 * c, 512),
                                start=(ps == 0 and tb == 0),
                                stop=(ps == 2 and tb == TB - 1),
                                perf_mode=DR)
                    nc.vector.tensor_scalar(
                        out=h8[:, ft, 512 * c:512 * c + 512], in0=acc[:],
                        scalar1=1.0 / 16.0, scalar2=0.0,
                        op0=Alu.mult, op1=Alu.max)

                ob_tiles = {}

                def outproj(s_prev, cts_prev, chunks, last=False):
                    for qt, ec in chunks:
                        acc = pout.tile([128, 512], F32, tag="out",
                                        name="opacc")
                        for p in range(4):
                            nc.tensor.matmul(
                                acc[:],
                                cts_prev[p][:, 128 * qt:128 * qt + 128],
                                wos[:, p, 512 * ec:512 * ec + 512],
                                start=(p == 0), stop=(p == 3))
                        if ec == 0:
                            ob_tiles[qt] = poba.tile([128, 1024], BF16,
                                                     tag="ob",
                                                     name=f"ob{s_prev}_{qt}")
                        ob = ob_tiles[qt]
                        if last and ec == 1 and qt == 3:
                            nc.scalar.copy(ob[:, 512 * ec:512 * ec + 512],
                                           acc[:])
                        else:
                            nc.vector.tensor_copy(
                                ob[:, 512 * ec:512 * ec + 512], acc[:])
                        if ec == 1:
                            qs0 = 512 * s_prev
                            nc.sync.dma_start(
                                attn_d[qs0 + 128 * qt:qs0 + 128 * qt + 128,
                                       :], ob[:])

                def nkt(s):
                    return 4 * (s + 1) if causal else LT

                cts_map = {}
                csb_map = {}
                ea_map = {}
                ktc = [0]

                pending = []     # deferred transpose+copy blocks

                def normalize(s, cps, h):
                    # cps [128, 4, 65]: per-chunk scale by 1/denominator
                    p = h // 2
                    if h % 2 == 0:
                        csb_map[(s, p)] = pcsb.tile([128, 4, 2, 64], BF16,
                                                    tag="csb",
                                                    name=f"csb{s}_{p}")
                    csb = csb_map[(s, p)]
                    rr = prr.tile([128, 4, 1], F32, tag="rr", name="rr")
                    nc.vector.reciprocal(rr[:], cps[:, :, 64:65])
                    for qc in range(4):
                        nc.vector.tensor_scalar_mul(
                            csb[:, qc, h % 2, :], cps[:, qc, 0:64],
                            rr[:, qc])
                    if h % 2 == 1:
                        # both heads of p normalized: defer the PE transposes
                        # so they don't stall on the DVE normalize above
                        pending.append((s, p, csb, cts_map[s]))
                        del csb_map[(s, p)]

                def finish_pair(force=False):
                    # depth-1 deferral only: outproj chunks read all four
                    # cts[p] tiles, so the last pair's transposes must land
                    # before the next supertile's first outproj chunk
                    if not pending:
                        return
                    s, p, csb, cts = pending.pop(0)
                    asm = pasm.tile([128, 512], BF16, tag="asm",
                                    name=f"asm{s}_{p}")
                    for qc in range(4):
                        nc.tensor.transpose(
                            asm[:, 128 * qc:128 * qc + 128],
                            csb[:, qc, :, :].rearrange("p a b -> p (a b)"),
                            ident[:])
                    nc.vector.tensor_copy(cts[p][:], asm[:])

                def produce_pair(s, h, kt0):
                    # two k-tiles (kt0, kt0+1) share one 2-bank psum tile so
                    # a single exp instruction covers both (halving the Act
                    # per-instruction overhead); the diagonal pairs keep two
                    # exps to skip the masked region
                    qs = 512 * s
                    g, u = divmod(h, 4)
                    psA = pps.tile([128, 2, 512], F32, tag="ps", name="sc")
                    ea = pexp.tile([128, 2, 512], BF16, tag="exp", name="ea")
                    offs = []
                    for j in range(2):
                        kt = kt0 + j
                        k0 = 128 * kt
                        off = min(max(0, k0 - qs), 384) if causal else 0
                        offs.append(off)
                        nc.tensor.matmul(
                            psA[:, j, off:512],
                            k8s[g][32 * u:32 * u + 32, :, k0:k0 + 128],
                            q8s[g][32 * u:32 * u + 32, :, qs + off:qs + 512],
                            start=True, stop=True,
                            perf_mode=DR, tile_position=(32 * u, 0))
                        ul = (min(max(k0 - BAND_OFF - qs, 0), 512)
                              if not causal else 0)
                        if ul > 0:
                            nc.vector.tensor_scalar_add(
                                psA[:, j, 0:ul], psA[:, j, 0:ul],
                                bfut_sb[:, h:h + 1])
                    psF = psA[:].rearrange("p a b -> p (a b)")
                    eaF = ea[:].rearrange("p a b -> p (a b)")
                    if offs[1] == 0:
                        nc.scalar.activation(eaF[:, 0:1024], psF[:, 0:1024],
                                             Act.Exp)
                    else:
                        nc.scalar.activation(eaF[:, offs[0]:512],
                                             psF[:, offs[0]:512], Act.Exp)
                        nc.scalar.activation(eaF[:, 512 + offs[1]:1024],
                                             psF[:, 512 + offs[1]:1024],
                                             Act.Exp)
                    for j in range(2):
                        kt = kt0 + j
                        k0 = 128 * kt
                        off = offs[j]
                        o_lo = max(k0 - BAND_OFF, qs + off)
                        o_hi = min(k0 + 256, qs + 512)
                        if o_hi > o_lo:
                            # exp(s+b) == exp(s)*exp(b): banded rel-pos bias
                            # (and causal-mask zeros) as a Pool multiply; the
                            # head's last pair goes on DVE (lower latency)
                            # since the drain waits on it soonest
                            psl = slice(o_lo - qs, o_hi - qs)
                            bsl = slice(o_lo - (k0 - BAND_OFF),
                                        o_hi - (k0 - BAND_OFF))
                            eng = (nc.vector if kt0 + 2 >= nkt(s)
                                   else nc.gpsimd)
                            eng.tensor_tensor(
                                ea[:, j, psl], ea[:, j, psl],
                                band3[:, h, bsl], Alu.mult)
                        ea_map[(s, h, kt)] = (ea[:, j], off)
                        ktc[0] += 1
                        if ktc[0] % 5 == 0:
                            emit_mlp_unit()

                def consume_head(s, h):
                    # drain the whole head qc-major: each query-chunk's psum
                    # accumulation group is contiguous (HW accumulate state
                    # is per-bank, so groups must not interleave)
                    cps = pctx.tile([128, 4, 65], F32, tag="ctx",
                                    name=f"cps{s}_{h}")
                    kmax = nkt(s) - 1
                    for qc in range(4):
                        lastk = min(kmax, 4 * s + qc) if causal else kmax
                        for kt in range(lastk + 1):
                            ea, off = ea_map[(s, h, kt)]
                            nc.tensor.matmul(
                                cps[:, qc, :],
                                ea[:, 128 * qc:128 * qc + 128],
                                va_all[:, 520 * kt + 65 * h:
                                       520 * kt + 65 * h + 65],
                                start=(kt == 0), stop=(kt == lastk))
                    for kt in range(nkt(s)):
                        ea_map.pop((s, h, kt))
                    normalize(s, cps, h)
                    if s > 0:
                        outproj(s - 1, cts_map[s - 1], [divmod(h, 2)])
                        if h == HC - 1:
                            del cts_map[s - 1]

                heads = [(s, h) for s in range(NSUP) for h in range(HC)]
                for j, (s, h) in enumerate(heads):
                    if h == 0:
                        cts_map[s] = [pct.tile([128, 512], BF16, tag="ct",
                                               name=f"ct{s}_{p}")
                                      for p in range(4)]
                    for kt0 in range(0, nkt(s), 2):
                        produce_pair(s, h, kt0)
                        if kt0 == 2:
                            finish_pair()
                            if j >= 1:
                                consume_head(*heads[j - 1])
                consume_head(*heads[-1])
                while pending:
                    finish_pair(force=True)
                outproj(NSUP - 1, cts_map[NSUP - 1],
                        [(qt, ec) for qt in range(4) for ec in range(2)],
                        last=True)
                while mlp_i < len(mlp_units):
                    emit_mlp_unit()

            # ---------------- MLP down-projection ---------------------------
            with (
                tc.tile_pool(name="pobb", bufs=2) as pobb,
                tc.tile_pool(name="pps3", bufs=3, space="PSUM") as pps3,
            ):
                h4 = h8[:].rearrange("p (j pb) l -> p j pb l", j=2)
                wm4 = [w[:].rearrange("p (j pb) e -> p j pb e", j=2)
                       for w in wms]
                for lt in range(LT):
                    ob = pobb.tile([128, 1024], BF16, tag="ob")
                    for ec in range(2):
                        # the very last unit splits into two independent
                        # 256-col accs so its first half's copy+DMA overlaps
                        # the second half's matmuls, shortening the final
                        # drain chain
                        nch = 2 if (lt == LT - 1 and ec == 1) else 1
                        w = 512 // nch
                        for ch in range(nch):
                            c0 = 512 * ec + ch * w
                            acc = pps3.tile([128, w], F32, tag="ps3",
                                            name="acc3")
                            for ps in range(2):
                                for pb in range(PB):
                                    nc.tensor.matmul(
                                        acc[:],
                                        h4[:, :, pb,
                                           128 * lt:128 * lt + 128],
                                        wm4[ps][:, :, pb, c0:c0 + w],
                                        start=(ps == 0 and pb == 0),
                                        stop=(ps == 1 and pb == PB - 1),
                                        perf_mode=DR)
                            nc.scalar.mul(ob[:, c0:c0 + w], acc[:],
                                          1.0 / 32.0)
                            nc.sync.dma_start(
                                mlp_d[128 * lt:128 * lt + 128, c0:c0 + w],
                                ob[:, c0:c0 + w])

    nc.compile()
    return nc


_NC_CACHE = {}


def _get_nc(causal: bool):
    if causal not in _NC_CACHE:
        _NC_CACHE[causal] = _build(causal)
    return _NC_CACHE[causal]


def _bucket(n):
    n = np.asarray(n)
    nf = np.maximum(n.astype(np.float32), np.float32(1.0))
    v = np.log(nf / np.float32(16.0)).astype(np.float32)
    v = (v / np.float32(np.log(8.0))) * np.float32(16.0)
    val_large = np.minimum(16 + v.astype(np.int32), NUM_BUCKETS - 1)
    return np.where(n < 16, n, val_large)


def _make_band(rel_emb, heads, causal):
    """exp() of the banded rel-pos bias (causal-masked entries -> 0)."""
    d = np.arange(-(BAND_OFF + 127), 256)
    pos = np.maximum(d, 0)
    bv = rel_emb[_bucket(pos)][:, heads] - rel_emb[NUM_BUCKETS - 1][heads]
    bv = np.where(d[:, None] >= 113, np.float32(0.0), bv)
    bv = np.exp(bv).astype(np.float32)
    if causal:
        bv = np.where(d[:, None] < 0, np.float32(0.0), bv)
    else:
        fut = np.exp(rel_emb[0][heads] - rel_emb[NUM_BUCKETS - 1][heads])
        bv = np.where(d[:, None] < 0, fut[None, :], bv)
    i = np.arange(128)[:, None]
    j = np.arange(BAND_W)[None, :]
    idx = (j - BAND_OFF - i) + (BAND_OFF + 127)
    return bv.astype(np.float32)[idx]          # [128, BAND_W, HC]


def _f8(a):
    return np.ascontiguousarray(a, dtype=np.float32).astype(E4)


def _split16(w, s):
    """-> (e4m3(s*w), e4m3(s*w - f32(e4m3(s*w))), e4m3(f32(e4m3(s*w))/s))"""
    w = np.asarray(w, np.float32)
    s1 = _f8(s * w)
    f1 = s1.astype(np.float32)
    s2 = _f8(s * w - f1)
    s3 = _f8(f1 / s)
    return s1, s2, s3


def _stat_qk(w_c):
    """w_c [E, HC, D] -> [4(tile), 128, TB, 2, 128] in f32 (pre-quant)."""
    arr = w_c.reshape(E, 2, 4, 2, 32)           # e, g, u, dp, dm
    out = np.empty((4, TB, 128, 2, 128), np.float32)
    for tl in range(4):
        g, dp = divmod(tl, 2)
        M = arr[:, g, :, dp, :].reshape(E, 128)  # m = 32u + dm
        out[tl] = M.reshape(TB, 2, 128, 128).transpose(0, 2, 1, 3)
    return out.transpose(0, 2, 1, 3, 4)          # [4, 128, TB, 2, 128]


def _prep_in_maps(inputs, wq, wk, wv, wo, wi, wmo, rel_emb, decoder_mask):
    inputs = np.asarray(inputs, dtype=np.float32)
    wq = np.asarray(wq, dtype=np.float32)
    wk = np.asarray(wk, dtype=np.float32)
    wv = np.asarray(wv, dtype=np.float32)
    wo = np.asarray(wo, dtype=np.float32)
    wi = np.asarray(wi, dtype=np.float32)
    wmo = np.asarray(wmo, dtype=np.float32)
    rel_emb = np.asarray(rel_emb, dtype=np.float32)
    mask = np.asarray(decoder_mask).reshape(L, L)

    tril = np.tril(np.ones((L, L), dtype=bool))
    if np.array_equal(mask, tril):
        causal = True
    elif mask.all():
        causal = False
    else:
        raise NotImplementedError("only causal or all-true masks supported")

    in_maps = []
    for c in range(NCORES):
        b, g = divmod(c, 2)
        heads = np.arange(HC * g, HC * (g + 1))
        band = _make_band(rel_emb, heads, causal)        # [128, W, HC]
        band = np.ascontiguousarray(band.transpose(0, 2, 1)).astype(BF)
        bfut = np.broadcast_to(
            (rel_emb[0][heads] - rel_emb[NUM_BUCKETS - 1][heads])
            .astype(np.float32), (128, HC)).copy()

        xT = inputs[b].T                                  # [E, L]
        x8 = _f8(xT)
        xr = _f8(16.0 * (xT - x8.astype(np.float32)))
        x8 = x8.reshape(ET, 128, L).transpose(1, 0, 2)    # [128, ET, L]
        xr = xr.reshape(ET, 128, L).transpose(1, 0, 2)

        wq_c = wq[:, heads, :]
        wk_c = wk[:, heads, :]
        # 2-pass q/k: keep only (s1, s3) -> [4, 128, 2, TB, 2, 128]
        q1, _, q3 = _split16(_stat_qk(wq_c), 16.0)
        k1, _, k3 = _split16(_stat_qk(wk_c), 16.0)
        wqs = np.ascontiguousarray(
            np.stack([q1, q3], axis=2))                  # [4,128,2,TB,2,128]
        wks = np.ascontiguousarray(np.stack([k1, k3], axis=2))

        wv_c = wv[:, heads, :].reshape(E, HC * D)
        wvm = np.empty((2, 128, 3, TB, 2, 256), E4)
        for vh in range(2):
            N = wv_c[:, 256 * vh:256 * vh + 256]
            N = N.reshape(TB, 2, 128, 256).transpose(2, 0, 1, 3)
            s1, s2, s3 = _split16(N, 16.0)
            wvm[vh, :, 0], wvm[vh, :, 1], wvm[vh, :, 2] = s1, s2, s3

        wi_c = wi[:, FC * g:FC * (g + 1)]
        wis = np.empty((FT // 2, 128, 2, 3, TB, 2, 128), E4)
        for ft in range(FT):
            M = wi_c[:, 128 * ft:128 * ft + 128]
            M = M.reshape(TB, 2, 128, 128).transpose(2, 0, 1, 3)
            s1, s2, s3 = _split16(M, 16.0)
            fp, j = divmod(ft, 2)
            wis[fp, :, j, 0], wis[fp, :, j, 1], wis[fp, :, j, 2] = s1, s2, s3

        wmo_c = wmo[FC * g:FC * (g + 1), :]               # [FC, E]
        wm = wmo_c.reshape(FT, 128, E).transpose(1, 0, 2)  # [128, FT, E]
        m1 = _f8(32.0 * wm)
        m2 = _f8(32.0 * wm - m1.astype(np.float32))
        wmm = np.stack([m1, m2])

        wo_c = wo[heads]                                   # [HC, D, E]
        wos = wo_c.reshape(4, 2, 64, E).transpose(0, 1, 2, 3) \
            .reshape(4, 128, E).transpose(1, 0, 2)         # [128, 4, E]
        wos = np.ascontiguousarray(wos).astype(BF)

        in_maps.append(dict(
            x8=np.ascontiguousarray(x8), xr=np.ascontiguousarray(xr),
            wqs=wqs, wks=wks, wvm=wvm, wis=wis, wmm=wmm,
            wos=wos, band=band, bfut=bfut,
            ident=np.eye(128, dtype=np.float32).astype(BF),
        ))
    return in_maps, causal, inputs


def run(trace=False, **kw):
    in_maps, causal, inputs = _prep_in_maps(**kw)
    nc = _get_nc(causal)
    res = run_bass_kernel_spmd(nc, in_maps, list(range(NCORES)), trace=trace)
    out = np.empty((B, L, E), dtype=np.float32)
    for b in range(B):
        out[b] = (inputs[b]
                  + res.results[2 * b]["attn_out"].astype(np.float32)
                  + res.results[2 * b]["mlp_out"].astype(np.float32)
                  + res.results[2 * b + 1]["attn_out"].astype(np.float32)
                  + res.results[2 * b + 1]["mlp_out"].astype(np.float32))
    return out, res


def kernel(**inputs):
    out, _ = run(**inputs)
    return out
